# revision 10
# baseline (speedup 1.0000x reference)
"""GeniePath (GAT breadth + LSTM depth) distributed Trainium2 Bass kernel.

Self-contained: takes FULL unsharded inputs as produced by
reference.setup_inputs(), returns the FULL [N, OUT_DIM] output.

Hardcoded problem shape:
  N=50000 nodes, E=800000 edges, IN_DIM=256, H=128, OUT_DIM=64, DEPTH=3.

Distribution: nodes (and their incoming edges, by dst) are sharded across
8 NeuronCores; weights are replicated. Per GAT layer each core computes the
rotated feature table y = h @ (W R) for its node shard (R is invertible
with first column attn_l, so el = y[:, 0] rides along with gathered rows),
AllGathers the table into every core's HBM, dma_gathers the per-edge source
rows (per (dst-block, src-half) calls whose valid-index counts are loaded
from a per-core table so padding descriptors are skipped), multiplies a
statically preloaded one-hot by the exp-weighted attention factors, and
contracts on the TensorEngine into per-destination aggregates + softmax
denominators. exp(er) factors are partition-replicated via a rank-1 matmul
(keeping the Pool engine free for gathers). The depth LSTM and output
projection are node-parallel. Edge bookkeeping (dst-sorted blocks of 64
nodes in block-major chunk order, low/high split so gather indices fit
int16) is precomputed on the host; all cores share one SPMD graph topology
(per-position chunk counts are maxed across cores).
"""

import numpy as np
import ml_dtypes

N = 50000
E = 800000
IN_DIM = 256
H = 128
OUT_DIM = 64
DEPTH = 3
NEG_SLOPE = 0.2

NCORES = 8
BLK = 64                      # dst nodes per block (one-hot width)
NODES_PC_RAW = N // NCORES    # 6250
NODES_PC = 6272               # = 49*128, padded per-core node count
NTILES = NODES_PC // 128      # 49
NBLK = NODES_PC // BLK        # 98
NTAB = NCORES * NODES_PC      # 50176 rows in the gathered table
LOW_CORES = 5
SPLIT = LOW_CORES * NODES_PC  # 31360 (< 32768 so low indices fit int16)
SEG_MAX_CHUNKS = 24

bf16 = ml_dtypes.bfloat16

_GRAPH = None
_PREP = None
RUN_KWARGS = {}      # test.py may set {"trace": True, "tmpdir": ...}
LAST_RESULT = None


def _preprocess(src, dst):
    """Host-side edge bookkeeping. Returns shared topology + per-core data."""
    src = np.asarray(src, np.int64)
    dst = np.asarray(dst, np.int64)
    core_of = np.minimum(dst // NODES_PC_RAW, NCORES - 1)

    per_core = []
    for c in range(NCORES):
        m = core_of == c
        s_c = src[m]
        d_c = dst[m] - c * NODES_PC_RAW
        lo = s_c // NODES_PC_RAW < LOW_CORES
        blk = d_c // BLK
        nL = np.bincount(blk[lo], minlength=NBLK)
        nH = np.bincount(blk[~lo], minlength=NBLK)
        pL = -(-nL // 128)
        pH = -(-nH // 128)
        per_core.append((s_c, d_c, lo, blk, pL, pH))

    orders = []
    for c in range(NCORES):
        pL, pH = per_core[c][4], per_core[c][5]
        orders.append(np.lexsort((-pL, -(pL + pH))))

    PL = np.zeros(NBLK, np.int64)
    PH = np.zeros(NBLK, np.int64)
    for c in range(NCORES):
        pL, pH = per_core[c][4], per_core[c][5]
        PL = np.maximum(PL, pL[orders[c]])
        PH = np.maximum(PH, pH[orders[c]])
    PL = np.maximum(PL, 1)
    PH = np.maximum(PH, 1)

    # segments: runs of equal (PL, PH), at most SEG_MAX_CHUNKS chunks each
    segments = []
    k = 0
    while k < NBLK:
        pl, ph = int(PL[k]), int(PH[k])
        assert pl + ph <= SEG_MAX_CHUNKS, (pl, ph)
        k2 = k
        while k2 < NBLK and PL[k2] == pl and PH[k2] == ph:
            k2 += 1
        per_seg = max(1, min(4, SEG_MAX_CHUNKS // (pl + ph)))
        kk = k
        while kk < k2:
            nb = int(min(per_seg, k2 - kk))
            segments.append((int(kk), nb, pl, ph))
            kk += nb
        k = k2

    # chunk layout: block-major — per block [L chunks | H chunks]
    seg_off = []
    tot = 0
    for (k0, nb, pl, ph) in segments:
        seg_off.append(tot)
        tot += nb * (pl + ph)
    CTOT = tot

    # gather call list: per (segment, block, part) split into <=8-chunk
    # pieces.  Shared across cores and layers.
    calls = []   # (chunk0, nchunks)
    for si, (k0, nb, pl, ph) in enumerate(segments):
        base = seg_off[si]
        for t in range(nb):
            for part, p_ in ((0, pl), (1, ph)):
                coff = base + t * (pl + ph) + (0 if part == 0 else pl)
                done = 0
                while done < p_:
                    n = int(min(8, p_ - done))
                    calls.append((coff + done, n))
                    done += n
    NCALLS = len(calls)

    # node relabeling perms
    perms = []
    for c in range(NCORES):
        perm = np.full(NODES_PC, -1, np.int64)
        order = orders[c]
        for pos in range(NBLK):
            b = order[pos]
            n0 = b * BLK
            n1 = min(n0 + BLK, NODES_PC_RAW)
            cnt = n1 - n0
            perm[pos * BLK: pos * BLK + cnt] = np.arange(n0, n1)
        perms.append(perm)
    inv_all = np.zeros((NCORES, NODES_PC_RAW), np.int64)
    for c in range(NCORES):
        pm = perms[c]
        valid = pm >= 0
        inv_all[c][pm[valid]] = np.nonzero(valid)[0]

    # per-core idx + one-hot + counts arrays in block-major chunk order
    cores_data = []
    for c in range(NCORES):
        s_c, d_c, lo, blk, _, _ = per_core[c]
        order = orders[c]
        idx_all = np.full(CTOT * 128, -1, np.int16)
        oh_all = np.zeros((128, CTOT, BLK), np.float32)
        counts = np.zeros(NCALLS, np.int32)

        sc_core = np.minimum(s_c // NODES_PC_RAW, NCORES - 1)
        s_gid = sc_core * NODES_PC + inv_all[sc_core, s_c - sc_core * NODES_PC_RAW]

        sort_key = np.lexsort((d_c, blk))
        s_gid_s = s_gid[sort_key]
        d_s = d_c[sort_key]
        lo_s = lo[sort_key]
        blk_s = blk[sort_key]
        blk_start = np.searchsorted(blk_s, np.arange(NBLK + 1))

        for si, (k0, nb, pl, ph) in enumerate(segments):
            base = seg_off[si]
            for t in range(nb):
                pos = k0 + t
                b = order[pos]
                sl = slice(blk_start[b], blk_start[b + 1])
                sg = s_gid_s[sl]
                dl = d_s[sl] - b * BLK
                lom = lo_s[sl]
                for part, p_ in ((0, pl), (1, ph)):
                    sel = lom if part == 0 else ~lom
                    sgx = sg[sel]
                    dlx = dl[sel]
                    n = len(sgx)
                    c0 = base + t * (pl + ph) + (0 if part == 0 else pl)
                    cap = 128 * p_
                    assert n <= cap, (c, pos, n, cap)
                    s0 = c0 * 128
                    vals = (sgx - (0 if part == 0 else SPLIT)).astype(np.int16)
                    idx_all[s0: s0 + n] = vals
                    j = np.arange(n)
                    oh_all[j % 128, c0 + j // 128, dlx] = 1.0
                    if n == 0:
                        idx_all[s0] = 0     # keep >=1 valid idx per part
        # per-call valid counts (>=1)
        for i, (c0, nch) in enumerate(calls):
            seg_idx = idx_all[c0 * 128: (c0 + nch) * 128]
            counts[i] = max(1, int((seg_idx >= 0).sum()))
            if (seg_idx >= 0).sum() == 0:
                idx_all[c0 * 128] = 0

        cores_data.append(dict(
            idx=idx_all, oh=oh_all.astype(bf16), counts=counts,
            perm=perms[c],
        ))

    topo = dict(segments=segments, seg_off=seg_off, CTOT=CTOT, calls=calls,
                NCALLS=NCALLS)
    return topo, cores_data


def _wrap_idx(a):
    """dma_gather idx layout: [128, n/16] — 16-wrap, replicated for 8 cores."""
    return np.tile(a.reshape(-1, 16).T.copy(), (8, 1))


def _rotation(a_l):
    """R [H,H] invertible with R[:,0] == a_l; returns (R, Rinv)."""
    a = np.asarray(a_l, np.float64)
    nrm = np.linalg.norm(a)
    v = a / nrm
    s = 1.0 if v[0] >= 0 else -1.0
    w = v.copy()
    w[0] += s
    u = w / np.linalg.norm(w)
    Hh = np.eye(H) - 2.0 * np.outer(u, u)
    R0 = -s * Hh
    Dv = np.ones(H)
    Dv[0] = nrm
    R = R0 * Dv[None, :]
    Rinv = (1.0 / Dv)[:, None] * R0.T
    return R.astype(np.float32), Rinv.astype(np.float32)


def _build_graph(topo, sim_mode=False, no_collective=False):
    import concourse.tile as tile
    from concourse import bacc, mybir

    BF = mybir.dt.bfloat16
    F32 = mybir.dt.float32
    I16 = mybir.dt.int16
    I32 = mybir.dt.int32
    AT = mybir.AluOpType
    AF = mybir.ActivationFunctionType

    segments = topo["segments"]
    seg_off = topo["seg_off"]
    CTOT = topo["CTOT"]
    calls = topo["calls"]
    NCALLS = topo["NCALLS"]

    nc = bacc.Bacc("TRN2", target_bir_lowering=False, debug=False,
                   num_devices=1 if sim_mode else NCORES,
                   num_swdge_queues=4)

    # external tensors (DRAM layout == SBUF layout, partition dim first)
    x_d = nc.dram_tensor("x", [128, 2, NODES_PC], BF, kind="ExternalInput")
    idx_d = nc.dram_tensor("idx", [128, CTOT * 8], I16, kind="ExternalInput")
    oh_d = nc.dram_tensor("oh01", [128, CTOT, BLK], BF, kind="ExternalInput")
    cnt_d = nc.dram_tensor("cnts", [1, NCALLS], I32, kind="ExternalInput")
    ident_d = nc.dram_tensor("ident", [128, 128], BF, kind="ExternalInput")
    wxw_d = nc.dram_tensor("wxw", [128, 2, H], BF, kind="ExternalInput")
    wxb_d = nc.dram_tensor("wxb", [128, 1], F32, kind="ExternalInput")
    wr_d = nc.dram_tensor("wr", [128, DEPTH, H], BF, kind="ExternalInput")
    rinv_d = nc.dram_tensor("rinv", [128, DEPTH, H], BF, kind="ExternalInput")
    varr_d = nc.dram_tensor("varr", [128, DEPTH, 128], BF, kind="ExternalInput")
    gatb_d = nc.dram_tensor("gatb", [128, DEPTH, 1], F32, kind="ExternalInput")
    gw_d = nc.dram_tensor("gw", [128, DEPTH * 8, 128], BF, kind="ExternalInput")
    gb_d = nc.dram_tensor("gb", [128, DEPTH * 4, 1], F32, kind="ExternalInput")
    outw_d = nc.dram_tensor("outw", [128, OUT_DIM], BF, kind="ExternalInput")
    outb_d = nc.dram_tensor("outb", [128, OUT_DIM], F32, kind="ExternalInput")
    out_d = nc.dram_tensor("out", [NODES_PC, OUT_DIM], F32,
                           kind="ExternalOutput")

    NCHUNK = [(i * 512, 512) for i in range(NODES_PC // 512)]
    if NODES_PC % 512:
        NCHUNK.append((NODES_PC - NODES_PC % 512, NODES_PC % 512))

    with tile.TileContext(nc) as tc:
        with (
            tc.tile_pool(name="sb", bufs=1) as sb,
            tc.tile_pool(name="ps", bufs=1, space="PSUM") as psp,
            tc.tile_pool(name="dram", bufs=2, space="DRAM") as dp,
        ):
            def load(dten, shape, dtype):
                t = sb.tile(shape, dtype, name=f"sb_{dten.name}")
                nc.sync.dma_start(t[:], dten.ap())
                return t

            idx_sb = load(idx_d, [128, CTOT * 8], I16)
            cnt_sb = load(cnt_d, [1, NCALLS], I32)
            ident_sb = load(ident_d, [128, 128], BF)
            wxw_sb = load(wxw_d, [128, 2, H], BF)
            wxb_sb = load(wxb_d, [128, 1], F32)
            wr_sb = load(wr_d, [128, DEPTH, H], BF)
            rinv_sb = load(rinv_d, [128, DEPTH, H], BF)
            varr_sb = load(varr_d, [128, DEPTH, 128], BF)
            gatb_sb = load(gatb_d, [128, DEPTH, 1], F32)
            gw_sb = load(gw_d, [128, DEPTH * 8, 128], BF)
            gb_sb = load(gb_d, [128, DEPTH * 4, 1], F32)
            outw_sb = load(outw_d, [128, OUT_DIM], BF)
            outb_sb = load(outb_d, [128, OUT_DIM], F32)
            ones_sb = sb.tile([128, 1], BF, name="ones")
            nc.vector.memset(ones_sb[:], 1.0)

            cst = sb.tile([128, NODES_PC], BF, name="cst")
            mu_bf = sb.tile([128, NODES_PC], BF, name="mu_bf")
            h1 = sb.tile([128, NODES_PC], BF, name="h1")
            h2 = sb.tile([128, NODES_PC], BF, name="h2")
            h0 = sb.tile([128, NODES_PC], BF, tag="h03", bufs=1)
            hcol = [h0, h1, h2, None]  # h3 allocated later from tag h03

            b_rep = sb.tile([128, NODES_PC], BF, name="b_rep")
            b2_rep = sb.tile([128, NODES_PC], BF, name="b2_rep")

            cnt_regs = [nc.gpsimd.alloc_register(f"gcnt{i}") for i in range(8)]

            # pre-zero gather buffers (skipped-pad slots read stale data; it
            # must be finite so 0 * stale == 0 in the aggregation matmul)
            GBUFS = 4
            for _ in range(GBUFS):
                gz = sb.tile([128, SEG_MAX_CHUNKS, 128], BF, tag="gath",
                             bufs=GBUFS)
                nc.vector.memset(gz[:], 0.0)

            def lstm_step(i, h_i):
                for n0, nn in NCHUNK:
                    gates = []
                    for m in range(4):
                        ps = psp.tile([128, 512], F32, tag="mmA", bufs=2)
                        for k, rhs in ((0, h_i), (1, mu_bf)):
                            nc.tensor.matmul(ps[:, 0:nn],
                                             gw_sb[:, (i * 2 + k) * 4 + m, :],
                                             rhs[:, n0:n0 + nn],
                                             start=(k == 0), stop=(k == 1))
                        gt = sb.tile([128, 512], F32, tag=f"gate{m}", bufs=1)
                        func = AF.Tanh if m == 3 else AF.Sigmoid
                        nc.scalar.activation(gt[:, 0:nn], ps[:, 0:nn], func,
                                             bias=gb_sb[:, i * 4 + m, :])
                        gates.append(gt)
                    ig, fg, og, ct = gates
                    nc.vector.tensor_mul(ig[:, 0:nn], ig[:, 0:nn], ct[:, 0:nn])
                    nc.vector.tensor_mul(fg[:, 0:nn], fg[:, 0:nn],
                                         cst[:, n0:n0 + nn])
                    nc.vector.tensor_add(cst[:, n0:n0 + nn], ig[:, 0:nn],
                                         fg[:, 0:nn])
                    nc.scalar.activation(ct[:, 0:nn], cst[:, n0:n0 + nn],
                                         AF.Tanh)
                    nc.vector.tensor_mul(mu_bf[:, n0:n0 + nn], og[:, 0:nn],
                                         ct[:, 0:nn])

            # ---- h0 = x @ wx_W + wx_b ----
            for n0, nn in NCHUNK:
                xc = sb.tile([128, 2, 512], BF, tag="xchunk", bufs=2)
                nc.sync.dma_start(xc[:, :, 0:nn], x_d.ap()[:, :, n0:n0 + nn])
                ps = psp.tile([128, 512], F32, tag="mmA", bufs=2)
                for k in range(2):
                    nc.tensor.matmul(ps[:, 0:nn], wxw_sb[:, k, :],
                                     xc[:, k, 0:nn],
                                     start=(k == 0), stop=(k == 1))
                nc.vector.tensor_scalar_add(mu_bf[:, n0:n0 + nn], ps[:, 0:nn],
                                            wxb_sb[:])
                nc.vector.tensor_scalar_add(hcol[0][:, n0:n0 + nn],
                                            ps[:, 0:nn], wxb_sb[:])
            nc.vector.memset(cst[:], 0.0)

            h_cur = hcol[0]

            for layer in range(DEPTH):
                # A: rotated table z = h @ (W R), node-major, to local DRAM
                ztab = dp.tile([NODES_PC, H], BF, name=f"ztab{layer}")
                for t in range(NTILES):
                    ps = psp.tile([128, 512], F32, tag="mmA", bufs=2)
                    nc.tensor.matmul(ps[:, 0:H],
                                     h_cur[:, t * 128:(t + 1) * 128],
                                     wr_sb[:, layer, :], start=True, stop=True)
                    zb = sb.tile([128, H], BF, tag="ztile", bufs=2)
                    nc.scalar.activation(zb[:], ps[:, 0:H], AF.Copy)
                    nc.sync.dma_start(ztab[t * 128:(t + 1) * 128, :], zb[:])

                # C: AllGather the table
                if sim_mode or no_collective:
                    ytab = dp.tile([NTAB, H], BF, name=f"ytab{layer}")
                    for cc in range(NCORES):
                        nc.sync.dma_start(
                            ytab[cc * NODES_PC:(cc + 1) * NODES_PC, :],
                            ztab[:])
                else:
                    ytab = dp.tile([NTAB, H], BF, name=f"ytab{layer}",
                                   addr_space="Shared")
                    nc.gpsimd.collective_compute(
                        "AllGather", AT.bypass,
                        replica_groups=[list(range(NCORES))],
                        ins=[ztab.opt()], outs=[ytab.opt()],
                    )

                # B: b = exp(er), b2 = exp(slope*er), partition-replicated
                # via a rank-1 matmul (varr rows are all equal to attn_r@W);
                # emitted after the collective so it runs underneath it
                for n0, nn in NCHUNK:
                    ps = psp.tile([128, 512], F32, tag="mmA", bufs=2)
                    nc.tensor.matmul(ps[:, 0:nn], varr_sb[:, layer, :],
                                     h_cur[:, n0:n0 + nn],
                                     start=True, stop=True)
                    nc.scalar.activation(b_rep[:, n0:n0 + nn],
                                         ps[:, 0:nn], AF.Exp)
                    nc.scalar.activation(b2_rep[:, n0:n0 + nn],
                                         ps[:, 0:nn], AF.Exp,
                                         scale=NEG_SLOPE)

                # LSTM step for the previous layer overlaps the collective
                if layer >= 1:
                    lstm_step(layer - 1, hcol[layer])

                if layer == DEPTH - 1:
                    h3 = sb.tile([128, NODES_PC], BF, tag="h03", bufs=1,
                                 name="h3")
                    hcol[3] = h3
                h_next = hcol[layer + 1]

                # D: edge phase
                gq = [0]
                call_i = [0]
                for si, (k0, nb, pl, ph) in enumerate(segments):
                    c0 = seg_off[si]
                    P = pl + ph
                    nch = nb * P
                    ohs = sb.tile([128, SEG_MAX_CHUNKS, BLK], BF,
                                  tag="oh01", bufs=3)
                    nc.sync.dma_start(ohs[:, 0:nch, :],
                                      oh_d.ap()[:, c0:c0 + nch, :])
                    g = sb.tile([128, SEG_MAX_CHUNKS, 128], BF,
                                tag="gath", bufs=GBUFS)
                    # collect this segment's gather calls, then batch-load
                    # their valid-index counts into registers in one go
                    seg_calls = []
                    for t in range(nb):
                        for part, p_, (tb0, tb1) in (
                            (0, pl, (0, SPLIT)),
                            (1, ph, (SPLIT, NTAB)),
                        ):
                            a0 = t * P + (0 if part == 0 else pl)
                            done = 0
                            while done < p_:
                                n = int(min(8, p_ - done))
                                ci = call_i[0]
                                assert calls[ci] == (c0 + a0 + done, n), (
                                    calls[ci], (c0 + a0 + done, n))
                                call_i[0] += 1
                                seg_calls.append((ci, a0 + done, n, tb0, tb1))
                                done += n
                    ci0 = seg_calls[0][0]
                    ncall = len(seg_calls)
                    assert ncall <= len(cnt_regs), ncall
                    assert seg_calls[-1][0] == ci0 + ncall - 1
                    nc.gpsimd.reg_load(cnt_regs[:ncall],
                                       cnt_sb[0:1, ci0:ci0 + ncall])
                    for (ci, a0, n, tb0, tb1) in seg_calls:
                        nc.gpsimd.dma_gather(
                            out_ap=g[:, a0:a0 + n, :],
                            in_ap=ytab[tb0:tb1, :],
                            idxs_ap=idx_sb[:, (c0 + a0) * 8:
                                           (c0 + a0 + n) * 8],
                            num_idxs=n * 128,
                            num_idxs_reg=cnt_regs[ci - ci0],
                            elem_size=H,
                            queue_num=gq[0],
                        )
                        gq[0] = (gq[0] + 1) % 4

                    a1 = sb.tile([128, SEG_MAX_CHUNKS], BF, tag="a1", bufs=3)
                    nc.scalar.activation(a1[:, 0:nch], g[:, 0:nch, 0], AF.Exp)
                    a2 = sb.tile([128, SEG_MAX_CHUNKS], BF, tag="a2", bufs=3)
                    nc.scalar.activation(a2[:, 0:nch], g[:, 0:nch, 0], AF.Exp,
                                         scale=NEG_SLOPE)

                    m1 = sb.tile([128, SEG_MAX_CHUNKS, BLK], BF,
                                 tag="m1", bufs=3)
                    m2 = sb.tile([128, SEG_MAX_CHUNKS, BLK], BF,
                                 tag="m2", bufs=2)
                    shp = [128, nb, P, BLK]
                    a1v = (a1[:, 0:nch].rearrange("p (nb q) -> p nb q", nb=nb)
                           .unsqueeze(3).broadcast_to(shp))
                    a2v = (a2[:, 0:nch].rearrange("p (nb q) -> p nb q", nb=nb)
                           .unsqueeze(3).broadcast_to(shp))
                    bv = (b_rep[:, k0 * BLK:(k0 + nb) * BLK]
                          .rearrange("p (nb v) -> p nb v", v=BLK)
                          .unsqueeze(2).broadcast_to(shp))
                    b2v = (b2_rep[:, k0 * BLK:(k0 + nb) * BLK]
                           .rearrange("p (nb v) -> p nb v", v=BLK)
                           .unsqueeze(2).broadcast_to(shp))
                    m1_4 = m1[:, 0:nch, :].rearrange(
                        "p (nb q) v -> p nb q v", nb=nb)
                    m2_4 = m2[:, 0:nch, :].rearrange(
                        "p (nb q) v -> p nb q v", nb=nb)
                    nc.vector.tensor_tensor(m1_4, a1v, bv, AT.mult)
                    nc.vector.tensor_tensor(m2_4, a2v, b2v, AT.mult)
                    nc.vector.tensor_tensor(m1[:, 0:nch, :], m1[:, 0:nch, :],
                                            m2[:, 0:nch, :], AT.max)
                    nc.vector.tensor_tensor(m1[:, 0:nch, :], m1[:, 0:nch, :],
                                            ohs[:, 0:nch, :], AT.mult)

                    # aggregate per block; blocks processed in pairs so the
                    # normalize/transpose/rinv tail runs at 128 width
                    t = 0
                    while t < nb:
                        npair = 2 if t + 1 < nb else 1
                        width = 64 * npair
                        aggp = psp.tile([128, H], F32, tag="agg", bufs=2)
                        denp = psp.tile([128, 1], F32, tag="den", bufs=2)
                        for u in range(npair):
                            for q in range(P):
                                ch = (t + u) * P + q
                                st = q == 0
                                sp_ = q == P - 1
                                nc.tensor.matmul(aggp[u * 64:(u + 1) * 64, :],
                                                 m1[:, ch, :], g[:, ch, :],
                                                 start=st, stop=sp_)
                                nc.tensor.matmul(denp[u * 64:(u + 1) * 64, :],
                                                 m1[:, ch, :], ones_sb[:],
                                                 start=st, stop=sp_)
                        deng = sb.tile([128, 1], F32, tag="deng", bufs=2)
                        nc.vector.tensor_scalar_max(deng[0:width], denp[0:width],
                                                    1e-16)
                        rden = sb.tile([128, 1], F32, tag="rden", bufs=2)
                        nc.vector.reciprocal(rden[0:width], deng[0:width])
                        ynorm = sb.tile([128, H], BF, tag="ynorm", bufs=2)
                        nc.vector.tensor_scalar_mul(ynorm[0:width, :],
                                                    aggp[0:width, :],
                                                    rden[0:width])
                        pt = psp.tile([128, 128], BF, tag="ptr", bufs=1)
                        nc.tensor.transpose(pt[:, 0:width], ynorm[0:width, :],
                                            ident_sb[0:width, 0:width])
                        ptsb = sb.tile([128, 128], BF, tag="ptsb", bufs=2)
                        nc.scalar.activation(ptsb[:, 0:width], pt[:, 0:width],
                                             AF.Copy)
                        pz = psp.tile([128, 128], F32, tag="pz", bufs=1)
                        nc.tensor.matmul(pz[:, 0:width], rinv_sb[:, layer, :],
                                         ptsb[:, 0:width],
                                         start=True, stop=True)
                        k = k0 + t
                        nc.scalar.activation(
                            h_next[:, k * BLK:k * BLK + width],
                            pz[:, 0:width], AF.Tanh,
                            bias=gatb_sb[:, layer, :])
                        t += npair

                assert call_i[0] == NCALLS, (call_i[0], NCALLS)
                h_cur = h_next

            lstm_step(DEPTH - 1, hcol[DEPTH])

            # output projection, node-major
            for t in range(NTILES):
                ps = psp.tile([128, 512], F32, tag="mmA", bufs=2)
                nc.tensor.matmul(ps[:, 0:OUT_DIM],
                                 mu_bf[:, t * 128:(t + 1) * 128],
                                 outw_sb[:], start=True, stop=True)
                ob = sb.tile([128, OUT_DIM], F32, tag="otile", bufs=2)
                nc.vector.tensor_add(ob[:], ps[:, 0:OUT_DIM], outb_sb[:])
                nc.vector.tensor_scalar_max(ob[:], ob[:], 0.0)
                nc.sync.dma_start(out_d.ap()[t * 128:(t + 1) * 128, :], ob[:])

    nc.compile()
    return nc


def kernel(x, src, dst, wx_W, wx_b, gat_W, gat_b, attn_l, attn_r,
           ig_W, ig_b, fg_W, fg_b, og_W, og_b, st_W, st_b, out_W, out_b):
    global _GRAPH, _PREP, LAST_RESULT
    from concourse.bass_utils import run_bass_kernel_spmd

    x = np.asarray(x, np.float32)
    src_i = np.asarray(src, np.int64)
    dst_i = np.asarray(dst, np.int64)

    key = (int(src_i[:100].sum()), int(dst_i[:100].sum()), len(src_i))
    if _PREP is None or _PREP[0] != key:
        topo, cores_data = _preprocess(src_i, dst_i)
        _PREP = (key, topo, cores_data)
    else:
        _, topo, cores_data = _PREP

    if _GRAPH is None:
        _GRAPH = _build_graph(topo)
    nc = _GRAPH

    wx_W = np.asarray(wx_W, np.float32)
    wx_b = np.asarray(wx_b, np.float32)
    gat_W = np.asarray(gat_W, np.float32)
    gat_b = np.asarray(gat_b, np.float32)
    attn_l = np.asarray(attn_l, np.float32)
    attn_r = np.asarray(attn_r, np.float32)
    out_W = np.asarray(out_W, np.float32)
    out_b = np.asarray(out_b, np.float32)

    wr = np.zeros((DEPTH, H, H), np.float32)
    rinv = np.zeros((DEPTH, H, H), np.float32)
    varr = np.zeros((DEPTH, H, 128), np.float32)
    for i in range(DEPTH):
        R, Ri = _rotation(attn_l[i])
        wr[i] = gat_W[i] @ R
        rinv[i] = Ri
        varr[i] = np.repeat((gat_W[i] @ attn_r[i])[:, None], 128, axis=1)

    # gw layout [128, DEPTH*8, 128]: [:, (i*2+k)*4+m, :] = W_m[i][k*128+p, :]
    gw = np.zeros((128, DEPTH * 8, 128), np.float32)
    gb = np.zeros((128, DEPTH * 4, 1), np.float32)
    for i in range(DEPTH):
        for m, (Wm, bm) in enumerate(((ig_W, ig_b), (fg_W, fg_b),
                                      (og_W, og_b), (st_W, st_b))):
            W = np.asarray(Wm, np.float32)[i]
            b = np.asarray(bm, np.float32)[i]
            for k in range(2):
                gw[:, (i * 2 + k) * 4 + m, :] = W[k * 128:(k + 1) * 128, :]
            gb[:, i * 4 + m, 0] = b

    shared = dict(
        ident=np.eye(128, dtype=np.float32).astype(bf16),
        wxw=np.ascontiguousarray(
            wx_W.reshape(2, 128, H).transpose(1, 0, 2)).astype(bf16),
        wxb=wx_b.reshape(128, 1),
        wr=np.ascontiguousarray(wr.transpose(1, 0, 2)).astype(bf16),
        rinv=np.ascontiguousarray(rinv.transpose(1, 0, 2)).astype(bf16),
        varr=np.ascontiguousarray(varr.transpose(1, 0, 2)).astype(bf16),
        gatb=np.ascontiguousarray(
            gat_b.reshape(DEPTH, 128, 1).transpose(1, 0, 2)),
        gw=gw.astype(bf16),
        gb=gb,
        outw=out_W.astype(bf16),
        outb=np.tile(out_b.reshape(1, OUT_DIM), (128, 1)).astype(np.float32),
    )

    in_maps = []
    for c in range(NCORES):
        cd = cores_data[c]
        perm = cd["perm"]
        xs = np.zeros((NODES_PC, IN_DIM), np.float32)
        valid = perm >= 0
        xs[valid] = x[c * NODES_PC_RAW + perm[valid]]
        m = dict(shared)
        # x layout [128, 2, NODES_PC]: [p, k, n] = x_fm[k*128+p, n]
        xt = np.ascontiguousarray(xs.T).reshape(2, 128, NODES_PC)
        m["x"] = np.ascontiguousarray(xt.transpose(1, 0, 2)).astype(bf16)
        m["idx"] = _wrap_idx(cd["idx"])
        m["oh01"] = cd["oh"]
        m["cnts"] = cd["counts"].reshape(1, -1)
        in_maps.append(m)

    res = run_bass_kernel_spmd(nc, in_maps, core_ids=list(range(NCORES)),
                               **RUN_KWARGS)
    LAST_RESULT = res

    out = np.zeros((N, OUT_DIM), np.float32)
    for c in range(NCORES):
        o = np.asarray(res.results[c]["out"], np.float32)
        perm = cores_data[c]["perm"]
        valid = perm >= 0
        out[c * NODES_PC_RAW + perm[valid]] = o[valid]
    return out


# revision 19
# speedup vs baseline: 1.0621x; 1.0621x over previous
"""GeniePath (GAT breadth + LSTM depth) distributed Trainium2 Bass kernel.

Self-contained: takes FULL unsharded inputs as produced by
reference.setup_inputs(), returns the FULL [N, OUT_DIM] output.

Hardcoded problem shape:
  N=50000 nodes, E=800000 edges, IN_DIM=256, H=128, OUT_DIM=64, DEPTH=3.

Distribution: nodes (and their incoming edges, by dst) are sharded across
8 NeuronCores; weights are replicated. Per GAT layer each core computes the
rotated feature table y = h @ (W R) for its node shard (R is invertible
with first column attn_l, so el = y[:, 0] rides along with gathered rows),
AllGathers the table into every core's HBM, dma_gathers the per-edge source
rows (per (dst-block, src-half) calls whose valid-index counts are loaded
from a per-core table so padding descriptors are skipped), multiplies a
statically preloaded one-hot by the exp-weighted attention factors, and
contracts on the TensorEngine into per-destination aggregates + softmax
denominators. exp(er) factors are partition-replicated via a rank-1 matmul
(keeping the Pool engine free for gathers). The depth LSTM and output
projection are node-parallel. Edge bookkeeping (dst-sorted blocks of 64
nodes in block-major chunk order, low/high split so gather indices fit
int16) is precomputed on the host; all cores share one SPMD graph topology
(per-position chunk counts are maxed across cores).
"""

import numpy as np
import ml_dtypes

N = 50000
E = 800000
IN_DIM = 256
H = 128
OUT_DIM = 64
DEPTH = 3
NEG_SLOPE = 0.2

NCORES = 8
BLK = 64                      # dst nodes per block (one-hot width)
NODES_PC_RAW = N // NCORES    # 6250
NODES_PC = 6272               # = 49*128, padded per-core node count
NTILES = NODES_PC // 128      # 49
NBLK = NODES_PC // BLK        # 98
NTAB = NCORES * NODES_PC      # 50176 rows in the gathered table
LOW_CORES = 5
SPLIT = LOW_CORES * NODES_PC  # 31360 (< 32768 so low indices fit int16)
SEG_MAX_CHUNKS = 32

bf16 = ml_dtypes.bfloat16

_GRAPH = None
_PREP = None
RUN_KWARGS = {}      # test.py may set {"trace": True, "tmpdir": ...}
LAST_RESULT = None


def _preprocess(src, dst):
    """Host-side edge bookkeeping. Returns shared topology + per-core data."""
    src = np.asarray(src, np.int64)
    dst = np.asarray(dst, np.int64)
    core_of = np.minimum(dst // NODES_PC_RAW, NCORES - 1)

    per_core = []
    for c in range(NCORES):
        m = core_of == c
        s_c = src[m]
        d_c = dst[m] - c * NODES_PC_RAW
        lo = s_c // NODES_PC_RAW < LOW_CORES
        blk = d_c // BLK
        nL = np.bincount(blk[lo], minlength=NBLK)
        nH = np.bincount(blk[~lo], minlength=NBLK)
        pL = -(-nL // 128)
        pH = -(-nH // 128)
        per_core.append((s_c, d_c, lo, blk, pL, pH))

    orders = []
    for c in range(NCORES):
        pL, pH = per_core[c][4], per_core[c][5]
        orders.append(np.lexsort((-pL, -(pL + pH))))

    PL = np.zeros(NBLK, np.int64)
    PH = np.zeros(NBLK, np.int64)
    for c in range(NCORES):
        pL, pH = per_core[c][4], per_core[c][5]
        PL = np.maximum(PL, pL[orders[c]])
        PH = np.maximum(PH, pH[orders[c]])
    PL = np.maximum(PL, 1)
    PH = np.maximum(PH, 1)

    # segments: runs of equal (PL, PH), at most SEG_MAX_CHUNKS chunks each
    segments = []
    k = 0
    while k < NBLK:
        pl, ph = int(PL[k]), int(PH[k])
        assert pl + ph <= SEG_MAX_CHUNKS, (pl, ph)
        k2 = k
        while k2 < NBLK and PL[k2] == pl and PH[k2] == ph:
            k2 += 1
        if (pl + ph) * 4 <= SEG_MAX_CHUNKS:
            per_seg = 4
        elif (pl + ph) * 2 <= SEG_MAX_CHUNKS:
            per_seg = 2
        else:
            per_seg = 1
        kk = k
        while kk < k2:
            nb = int(min(per_seg, k2 - kk))
            segments.append((int(kk), nb, pl, ph))
            kk += nb
        k = k2

    # chunk layout: block-major — per block [L chunks | H chunks]
    seg_off = []
    tot = 0
    for (k0, nb, pl, ph) in segments:
        seg_off.append(tot)
        tot += nb * (pl + ph)
    CTOT = tot

    # gather call list: per (segment, block, part) split into <=8-chunk
    # pieces.  Shared across cores and layers.
    calls = []   # (chunk0, nchunks)
    for si, (k0, nb, pl, ph) in enumerate(segments):
        base = seg_off[si]
        for t in range(nb):
            for part, p_ in ((0, pl), (1, ph)):
                coff = base + t * (pl + ph) + (0 if part == 0 else pl)
                done = 0
                while done < p_:
                    n = int(min(8, p_ - done))
                    calls.append((coff + done, n))
                    done += n
    NCALLS = len(calls)

    # node relabeling perms
    perms = []
    for c in range(NCORES):
        perm = np.full(NODES_PC, -1, np.int64)
        order = orders[c]
        for pos in range(NBLK):
            b = order[pos]
            n0 = b * BLK
            n1 = min(n0 + BLK, NODES_PC_RAW)
            cnt = n1 - n0
            perm[pos * BLK: pos * BLK + cnt] = np.arange(n0, n1)
        perms.append(perm)
    inv_all = np.zeros((NCORES, NODES_PC_RAW), np.int64)
    for c in range(NCORES):
        pm = perms[c]
        valid = pm >= 0
        inv_all[c][pm[valid]] = np.nonzero(valid)[0]

    # per-core idx + one-hot + counts arrays in block-major chunk order
    cores_data = []
    for c in range(NCORES):
        s_c, d_c, lo, blk, _, _ = per_core[c]
        order = orders[c]
        idx_all = np.full(CTOT * 128, -1, np.int16)
        oh_all = np.zeros((128, CTOT, BLK), np.float32)
        counts = np.zeros(NCALLS, np.int32)

        sc_core = np.minimum(s_c // NODES_PC_RAW, NCORES - 1)
        s_gid = sc_core * NODES_PC + inv_all[sc_core, s_c - sc_core * NODES_PC_RAW]

        sort_key = np.lexsort((d_c, blk))
        s_gid_s = s_gid[sort_key]
        d_s = d_c[sort_key]
        lo_s = lo[sort_key]
        blk_s = blk[sort_key]
        blk_start = np.searchsorted(blk_s, np.arange(NBLK + 1))

        for si, (k0, nb, pl, ph) in enumerate(segments):
            base = seg_off[si]
            for t in range(nb):
                pos = k0 + t
                b = order[pos]
                sl = slice(blk_start[b], blk_start[b + 1])
                sg = s_gid_s[sl]
                dl = d_s[sl] - b * BLK
                lom = lo_s[sl]
                for part, p_ in ((0, pl), (1, ph)):
                    sel = lom if part == 0 else ~lom
                    sgx = sg[sel]
                    dlx = dl[sel]
                    n = len(sgx)
                    c0 = base + t * (pl + ph) + (0 if part == 0 else pl)
                    cap = 128 * p_
                    assert n <= cap, (c, pos, n, cap)
                    s0 = c0 * 128
                    vals = (sgx - (0 if part == 0 else SPLIT)).astype(np.int16)
                    idx_all[s0: s0 + n] = vals
                    j = np.arange(n)
                    oh_all[j % 128, c0 + j // 128, dlx] = 1.0
                    if n == 0:
                        idx_all[s0] = 0     # keep >=1 valid idx per part
        # per-call valid counts (>=1)
        for i, (c0, nch) in enumerate(calls):
            seg_idx = idx_all[c0 * 128: (c0 + nch) * 128]
            counts[i] = max(1, int((seg_idx >= 0).sum()))
            if (seg_idx >= 0).sum() == 0:
                idx_all[c0 * 128] = 0

        cores_data.append(dict(
            idx=idx_all, oh=oh_all.astype(bf16), counts=counts,
            perm=perms[c],
        ))

    topo = dict(segments=segments, seg_off=seg_off, CTOT=CTOT, calls=calls,
                NCALLS=NCALLS)
    return topo, cores_data


def _wrap_idx(a):
    """dma_gather idx layout: [128, n/16] — 16-wrap, replicated for 8 cores."""
    return np.tile(a.reshape(-1, 16).T.copy(), (8, 1))


def _rotation(a_l):
    """R [H,H] invertible with R[:,0] == a_l; returns (R, Rinv)."""
    a = np.asarray(a_l, np.float64)
    nrm = np.linalg.norm(a)
    v = a / nrm
    s = 1.0 if v[0] >= 0 else -1.0
    w = v.copy()
    w[0] += s
    u = w / np.linalg.norm(w)
    Hh = np.eye(H) - 2.0 * np.outer(u, u)
    R0 = -s * Hh
    Dv = np.ones(H)
    Dv[0] = nrm
    R = R0 * Dv[None, :]
    Rinv = (1.0 / Dv)[:, None] * R0.T
    return R.astype(np.float32), Rinv.astype(np.float32)


def _build_graph(topo, sim_mode=False, no_collective=False):
    import concourse.tile as tile
    from concourse import bacc, mybir

    BF = mybir.dt.bfloat16
    F32 = mybir.dt.float32
    I16 = mybir.dt.int16
    I32 = mybir.dt.int32
    AT = mybir.AluOpType
    AF = mybir.ActivationFunctionType

    segments = topo["segments"]
    seg_off = topo["seg_off"]
    CTOT = topo["CTOT"]
    calls = topo["calls"]
    NCALLS = topo["NCALLS"]

    nc = bacc.Bacc("TRN2", target_bir_lowering=False, debug=False,
                   num_devices=1 if sim_mode else NCORES,
                   num_swdge_queues=4)

    # external tensors (DRAM layout == SBUF layout, partition dim first)
    x_d = nc.dram_tensor("x", [128, 2, NODES_PC], BF, kind="ExternalInput")
    idx_d = nc.dram_tensor("idx", [128, CTOT * 8], I16, kind="ExternalInput")
    oh_d = nc.dram_tensor("oh01", [128, CTOT, BLK], BF, kind="ExternalInput")
    cnt_d = nc.dram_tensor("cnts", [1, NCALLS], I32, kind="ExternalInput")
    ident_d = nc.dram_tensor("ident", [128, 128], BF, kind="ExternalInput")
    wxw_d = nc.dram_tensor("wxw", [128, 2, H], BF, kind="ExternalInput")
    wxb_d = nc.dram_tensor("wxb", [128, 1], F32, kind="ExternalInput")
    wr_d = nc.dram_tensor("wr", [128, DEPTH, H], BF, kind="ExternalInput")
    rinv_d = nc.dram_tensor("rinv", [128, DEPTH, H], BF, kind="ExternalInput")
    varr_d = nc.dram_tensor("varr", [128, DEPTH, 128], BF, kind="ExternalInput")
    gatb_d = nc.dram_tensor("gatb", [128, DEPTH, 1], F32, kind="ExternalInput")
    gw_d = nc.dram_tensor("gw", [128, DEPTH * 8, 128], BF, kind="ExternalInput")
    gb_d = nc.dram_tensor("gb", [128, DEPTH * 4, 1], F32, kind="ExternalInput")
    outw_d = nc.dram_tensor("outw", [128, OUT_DIM], BF, kind="ExternalInput")
    outb_d = nc.dram_tensor("outb", [128, OUT_DIM], F32, kind="ExternalInput")
    out_d = nc.dram_tensor("out", [NODES_PC, OUT_DIM], F32,
                           kind="ExternalOutput")

    NCHUNK = [(i * 512, 512) for i in range(NODES_PC // 512)]
    if NODES_PC % 512:
        NCHUNK.append((NODES_PC - NODES_PC % 512, NODES_PC % 512))

    with tile.TileContext(nc) as tc:
        with (
            tc.tile_pool(name="sb", bufs=1) as sb,
            tc.tile_pool(name="ps", bufs=1, space="PSUM") as psp,
            tc.tile_pool(name="dram", bufs=2, space="DRAM") as dp,
        ):
            def load(dten, shape, dtype):
                t = sb.tile(shape, dtype, name=f"sb_{dten.name}")
                nc.sync.dma_start(t[:], dten.ap())
                return t

            idx_sb = load(idx_d, [128, CTOT * 8], I16)
            cnt_sb = load(cnt_d, [1, NCALLS], I32)
            ident_sb = load(ident_d, [128, 128], BF)
            wxw_sb = load(wxw_d, [128, 2, H], BF)
            wxb_sb = load(wxb_d, [128, 1], F32)
            wr_sb = load(wr_d, [128, DEPTH, H], BF)
            rinv_sb = load(rinv_d, [128, DEPTH, H], BF)
            varr_sb = load(varr_d, [128, DEPTH, 128], BF)
            gatb_sb = load(gatb_d, [128, DEPTH, 1], F32)
            gw_sb = load(gw_d, [128, DEPTH * 8, 128], BF)
            gb_sb = load(gb_d, [128, DEPTH * 4, 1], F32)
            outw_sb = load(outw_d, [128, OUT_DIM], BF)
            outb_sb = load(outb_d, [128, OUT_DIM], F32)
            ones_sb = sb.tile([128, 1], BF, name="ones")
            nc.vector.memset(ones_sb[:], 1.0)

            cst = sb.tile([128, NODES_PC], BF, name="cst")
            mu_bf = sb.tile([128, NODES_PC], BF, name="mu_bf")
            h1 = sb.tile([128, NODES_PC], BF, name="h1")
            h2 = sb.tile([128, NODES_PC], BF, name="h2")
            h0 = sb.tile([128, NODES_PC], BF, tag="h03", bufs=1)
            hcol = [h0, h1, h2, None]  # h3 allocated later from tag h03

            b_rep = sb.tile([128, NODES_PC], BF, name="b_rep")
            b2_rep = sb.tile([128, NODES_PC], BF, name="b2_rep")

            cnt_regs = [nc.gpsimd.alloc_register(f"gcnt{i}") for i in range(8)]

            # pre-zero gather buffers (skipped-pad slots read stale data; it
            # must be finite so 0 * stale == 0 in the aggregation matmul)
            GBUFS = 4
            NSEG = len(segments)
            for _ in range(GBUFS):
                gz = sb.tile([128, SEG_MAX_CHUNKS, 128], BF, tag="gath",
                             bufs=GBUFS)
                nc.vector.memset(gz[:], 0.0)

            def lstm_step(i, h_i, chunks=None):
                for n0, nn in (NCHUNK if chunks is None else chunks):
                    gates = []
                    for m in range(4):
                        ps = psp.tile([128, 512], F32, tag="mmA", bufs=2)
                        for k, rhs in ((0, h_i), (1, mu_bf)):
                            nc.tensor.matmul(ps[:, 0:nn],
                                             gw_sb[:, (i * 2 + k) * 4 + m, :],
                                             rhs[:, n0:n0 + nn],
                                             start=(k == 0), stop=(k == 1))
                        gt = sb.tile([128, 512], F32, tag=f"gate{m}", bufs=1)
                        func = AF.Tanh if m == 3 else AF.Sigmoid
                        nc.scalar.activation(gt[:, 0:nn], ps[:, 0:nn], func,
                                             bias=gb_sb[:, i * 4 + m, :])
                        gates.append(gt)
                    ig, fg, og, ct = gates
                    nc.vector.tensor_mul(ig[:, 0:nn], ig[:, 0:nn], ct[:, 0:nn])
                    nc.vector.tensor_mul(fg[:, 0:nn], fg[:, 0:nn],
                                         cst[:, n0:n0 + nn])
                    nc.vector.tensor_add(cst[:, n0:n0 + nn], ig[:, 0:nn],
                                         fg[:, 0:nn])
                    nc.scalar.activation(ct[:, 0:nn], cst[:, n0:n0 + nn],
                                         AF.Tanh)
                    nc.vector.tensor_mul(mu_bf[:, n0:n0 + nn], og[:, 0:nn],
                                         ct[:, 0:nn])

            def out_proj(chunks):
                # final projection for node columns covered by `chunks`
                for n0, nn in chunks:
                    for t0 in range(n0, n0 + nn, 128):
                        ps = psp.tile([128, 512], F32, tag="mmA", bufs=2)
                        nc.tensor.matmul(ps[:, 0:OUT_DIM],
                                         mu_bf[:, t0:t0 + 128],
                                         outw_sb[:], start=True, stop=True)
                        ob = sb.tile([128, OUT_DIM], F32, tag="otile", bufs=2)
                        nc.vector.tensor_add(ob[:], ps[:, 0:OUT_DIM],
                                             outb_sb[:])
                        nc.vector.tensor_scalar_max(ob[:], ob[:], 0.0)
                        nc.sync.dma_start(out_d.ap()[t0:t0 + 128, :], ob[:])

            # ---- h0 = x @ wx_W + wx_b ----
            for n0, nn in NCHUNK:
                xc = sb.tile([128, 2, 512], BF, tag="xchunk", bufs=2)
                nc.sync.dma_start(xc[:, :, 0:nn], x_d.ap()[:, :, n0:n0 + nn])
                ps = psp.tile([128, 512], F32, tag="mmA", bufs=2)
                for k in range(2):
                    nc.tensor.matmul(ps[:, 0:nn], wxw_sb[:, k, :],
                                     xc[:, k, 0:nn],
                                     start=(k == 0), stop=(k == 1))
                nc.vector.tensor_scalar_add(mu_bf[:, n0:n0 + nn], ps[:, 0:nn],
                                            wxb_sb[:])
                nc.vector.tensor_scalar_add(hcol[0][:, n0:n0 + nn],
                                            ps[:, 0:nn], wxb_sb[:])
            nc.vector.memset(cst[:], 0.0)

            h_cur = hcol[0]

            for layer in range(DEPTH):
                # A: rotated table z = h @ (W R), node-major, to local DRAM
                ztab = dp.tile([NODES_PC, H], BF, name=f"ztab{layer}")
                for t in range(NTILES):
                    ps = psp.tile([128, 512], F32, tag="mmA", bufs=2)
                    nc.tensor.matmul(ps[:, 0:H],
                                     h_cur[:, t * 128:(t + 1) * 128],
                                     wr_sb[:, layer, :], start=True, stop=True)
                    zb = sb.tile([128, H], BF, tag="ztile", bufs=2)
                    nc.scalar.activation(zb[:], ps[:, 0:H], AF.Copy)
                    nc.sync.dma_start(ztab[t * 128:(t + 1) * 128, :], zb[:])

                # C: AllGather the table
                if sim_mode or no_collective:
                    ytab = dp.tile([NTAB, H], BF, name=f"ytab{layer}")
                    for cc in range(NCORES):
                        nc.sync.dma_start(
                            ytab[cc * NODES_PC:(cc + 1) * NODES_PC, :],
                            ztab[:])
                else:
                    ytab = dp.tile([NTAB, H], BF, name=f"ytab{layer}",
                                   addr_space="Shared")
                    nc.gpsimd.collective_compute(
                        "AllGather", AT.bypass,
                        replica_groups=[list(range(NCORES))],
                        ins=[ztab.opt()], outs=[ytab.opt()],
                    )

                # B: b = exp(er), b2 = exp(slope*er), partition-replicated
                # via a rank-1 matmul (varr rows are all equal to attn_r@W);
                # emitted after the collective so it runs underneath it
                for n0, nn in NCHUNK:
                    ps = psp.tile([128, 512], F32, tag="mmA", bufs=2)
                    nc.tensor.matmul(ps[:, 0:nn], varr_sb[:, layer, :],
                                     h_cur[:, n0:n0 + nn],
                                     start=True, stop=True)
                    nc.scalar.activation(b_rep[:, n0:n0 + nn],
                                         ps[:, 0:nn], AF.Exp)
                    nc.scalar.activation(b2_rep[:, n0:n0 + nn],
                                         ps[:, 0:nn], AF.Exp,
                                         scale=NEG_SLOPE)

                # LSTM step for the previous layer overlaps the collective
                if layer >= 1:
                    lstm_step(layer - 1, hcol[layer])

                if layer == DEPTH - 1:
                    h3 = sb.tile([128, NODES_PC], BF, tag="h03", bufs=1,
                                 name="h3")
                    hcol[3] = h3
                h_next = hcol[layer + 1]

                # D: edge phase
                gq = [0]
                call_i = [0]
                done_cols = 0     # h_next columns finished (for interleave)
                lstm_cols = 0     # columns already pushed through final lstm
                for si, (k0, nb, pl, ph) in enumerate(segments):
                    c0 = seg_off[si]
                    P = pl + ph
                    nch = nb * P
                    ohs = sb.tile([128, SEG_MAX_CHUNKS, BLK], BF,
                                  tag="oh01", bufs=3)
                    nc.sync.dma_start(ohs[:, 0:nch, :],
                                      oh_d.ap()[:, c0:c0 + nch, :])
                    g = sb.tile([128, SEG_MAX_CHUNKS, 128], BF,
                                tag="gath", bufs=GBUFS)
                    # collect this segment's gather calls, then batch-load
                    # their valid-index counts into registers in one go
                    seg_calls = []
                    for t in range(nb):
                        for part, p_, (tb0, tb1) in (
                            (0, pl, (0, SPLIT)),
                            (1, ph, (SPLIT, NTAB)),
                        ):
                            a0 = t * P + (0 if part == 0 else pl)
                            done = 0
                            while done < p_:
                                n = int(min(8, p_ - done))
                                ci = call_i[0]
                                assert calls[ci] == (c0 + a0 + done, n), (
                                    calls[ci], (c0 + a0 + done, n))
                                call_i[0] += 1
                                seg_calls.append((ci, a0 + done, n, tb0, tb1))
                                done += n
                    ci0 = seg_calls[0][0]
                    ncall = len(seg_calls)
                    assert ncall <= len(cnt_regs), ncall
                    assert seg_calls[-1][0] == ci0 + ncall - 1
                    nc.gpsimd.reg_load(cnt_regs[:ncall],
                                       cnt_sb[0:1, ci0:ci0 + ncall])
                    for (ci, a0, n, tb0, tb1) in seg_calls:
                        nc.gpsimd.dma_gather(
                            out_ap=g[:, a0:a0 + n, :],
                            in_ap=ytab[tb0:tb1, :],
                            idxs_ap=idx_sb[:, (c0 + a0) * 8:
                                           (c0 + a0 + n) * 8],
                            num_idxs=n * 128,
                            num_idxs_reg=cnt_regs[ci - ci0],
                            elem_size=H,
                            queue_num=gq[0],
                        )
                        gq[0] = (gq[0] + 1) % 4

                    a1 = sb.tile([128, SEG_MAX_CHUNKS], BF, tag="a1", bufs=3)
                    nc.scalar.activation(a1[:, 0:nch], g[:, 0:nch, 0], AF.Exp)
                    a2 = sb.tile([128, SEG_MAX_CHUNKS], BF, tag="a2", bufs=3)
                    nc.scalar.activation(a2[:, 0:nch], g[:, 0:nch, 0], AF.Exp,
                                         scale=NEG_SLOPE)

                    m1 = sb.tile([128, SEG_MAX_CHUNKS, BLK], BF,
                                 tag="m1", bufs=3)
                    m2 = sb.tile([128, SEG_MAX_CHUNKS, BLK], BF,
                                 tag="m2", bufs=2)
                    shp = [128, nb, P, BLK]
                    a1v = (a1[:, 0:nch].rearrange("p (nb q) -> p nb q", nb=nb)
                           .unsqueeze(3).broadcast_to(shp))
                    a2v = (a2[:, 0:nch].rearrange("p (nb q) -> p nb q", nb=nb)
                           .unsqueeze(3).broadcast_to(shp))
                    bv = (b_rep[:, k0 * BLK:(k0 + nb) * BLK]
                          .rearrange("p (nb v) -> p nb v", v=BLK)
                          .unsqueeze(2).broadcast_to(shp))
                    b2v = (b2_rep[:, k0 * BLK:(k0 + nb) * BLK]
                           .rearrange("p (nb v) -> p nb v", v=BLK)
                           .unsqueeze(2).broadcast_to(shp))
                    m1_4 = m1[:, 0:nch, :].rearrange(
                        "p (nb q) v -> p nb q v", nb=nb)
                    m2_4 = m2[:, 0:nch, :].rearrange(
                        "p (nb q) v -> p nb q v", nb=nb)
                    nc.vector.tensor_tensor(m1_4, a1v, bv, AT.mult)
                    nc.vector.tensor_tensor(m2_4, a2v, b2v, AT.mult)
                    nc.vector.tensor_tensor(m1[:, 0:nch, :], m1[:, 0:nch, :],
                                            m2[:, 0:nch, :], AT.max)
                    nc.vector.tensor_tensor(m1[:, 0:nch, :], m1[:, 0:nch, :],
                                            ohs[:, 0:nch, :], AT.mult)

                    # aggregate per block; blocks processed in pairs so the
                    # normalize/transpose/rinv tail runs at 128 width
                    t = 0
                    while t < nb:
                        npair = 2 if t + 1 < nb else 1
                        width = 64 * npair
                        aggp = psp.tile([128, H], F32, tag="agg", bufs=2)
                        denp = psp.tile([128, 1], F32, tag="den", bufs=2)
                        for u in range(npair):
                            for q in range(P):
                                ch = (t + u) * P + q
                                st = q == 0
                                sp_ = q == P - 1
                                nc.tensor.matmul(aggp[u * 64:(u + 1) * 64, :],
                                                 m1[:, ch, :], g[:, ch, :],
                                                 start=st, stop=sp_)
                                nc.tensor.matmul(denp[u * 64:(u + 1) * 64, :],
                                                 m1[:, ch, :], ones_sb[:],
                                                 start=st, stop=sp_)
                        deng = sb.tile([128, 1], F32, tag="deng", bufs=2)
                        nc.vector.tensor_scalar_max(deng[0:width], denp[0:width],
                                                    1e-16)
                        rden = sb.tile([128, 1], F32, tag="rden", bufs=2)
                        nc.vector.reciprocal(rden[0:width], deng[0:width])
                        ynorm = sb.tile([128, H], BF, tag="ynorm", bufs=2)
                        nc.vector.tensor_scalar_mul(ynorm[0:width, :],
                                                    aggp[0:width, :],
                                                    rden[0:width])
                        pt = psp.tile([128, 128], BF, tag="ptr", bufs=1)
                        nc.tensor.transpose(pt[:, 0:width], ynorm[0:width, :],
                                            ident_sb[0:width, 0:width])
                        ptsb = sb.tile([128, 128], BF, tag="ptsb", bufs=2)
                        nc.scalar.activation(ptsb[:, 0:width], pt[:, 0:width],
                                             AF.Copy)
                        pz = psp.tile([128, 128], F32, tag="pz", bufs=1)
                        nc.tensor.matmul(pz[:, 0:width], rinv_sb[:, layer, :],
                                         ptsb[:, 0:width],
                                         start=True, stop=True)
                        k = k0 + t
                        nc.scalar.activation(
                            h_next[:, k * BLK:k * BLK + width],
                            pz[:, 0:width], AF.Tanh,
                            bias=gatb_sb[:, layer, :])
                        t += npair

                    # interleave the final LSTM step + output projection with
                    # the last layer's edge phase, per finished 512-col chunk
                    if layer == DEPTH - 1:
                        done_cols = (k0 + nb) * BLK
                        while lstm_cols + 512 <= done_cols:
                            ck = [(lstm_cols, 512)]
                            lstm_step(DEPTH - 1, h_next, chunks=ck)
                            out_proj(ck)
                            lstm_cols += 512

                assert call_i[0] == NCALLS, (call_i[0], NCALLS)
                h_cur = h_next

            if lstm_cols < NODES_PC:
                ck = [(lstm_cols, NODES_PC - lstm_cols)]
                lstm_step(DEPTH - 1, hcol[DEPTH], chunks=ck)
                out_proj(ck)

    nc.compile()
    return nc


def kernel(x, src, dst, wx_W, wx_b, gat_W, gat_b, attn_l, attn_r,
           ig_W, ig_b, fg_W, fg_b, og_W, og_b, st_W, st_b, out_W, out_b):
    global _GRAPH, _PREP, LAST_RESULT
    from concourse.bass_utils import run_bass_kernel_spmd

    x = np.asarray(x, np.float32)
    src_i = np.asarray(src, np.int64)
    dst_i = np.asarray(dst, np.int64)

    key = (int(src_i[:100].sum()), int(dst_i[:100].sum()), len(src_i))
    if _PREP is None or _PREP[0] != key:
        topo, cores_data = _preprocess(src_i, dst_i)
        _PREP = (key, topo, cores_data)
    else:
        _, topo, cores_data = _PREP

    if _GRAPH is None:
        _GRAPH = _build_graph(topo)
    nc = _GRAPH

    wx_W = np.asarray(wx_W, np.float32)
    wx_b = np.asarray(wx_b, np.float32)
    gat_W = np.asarray(gat_W, np.float32)
    gat_b = np.asarray(gat_b, np.float32)
    attn_l = np.asarray(attn_l, np.float32)
    attn_r = np.asarray(attn_r, np.float32)
    out_W = np.asarray(out_W, np.float32)
    out_b = np.asarray(out_b, np.float32)

    wr = np.zeros((DEPTH, H, H), np.float32)
    rinv = np.zeros((DEPTH, H, H), np.float32)
    varr = np.zeros((DEPTH, H, 128), np.float32)
    for i in range(DEPTH):
        R, Ri = _rotation(attn_l[i])
        wr[i] = gat_W[i] @ R
        rinv[i] = Ri
        varr[i] = np.repeat((gat_W[i] @ attn_r[i])[:, None], 128, axis=1)

    # gw layout [128, DEPTH*8, 128]: [:, (i*2+k)*4+m, :] = W_m[i][k*128+p, :]
    gw = np.zeros((128, DEPTH * 8, 128), np.float32)
    gb = np.zeros((128, DEPTH * 4, 1), np.float32)
    for i in range(DEPTH):
        for m, (Wm, bm) in enumerate(((ig_W, ig_b), (fg_W, fg_b),
                                      (og_W, og_b), (st_W, st_b))):
            W = np.asarray(Wm, np.float32)[i]
            b = np.asarray(bm, np.float32)[i]
            for k in range(2):
                gw[:, (i * 2 + k) * 4 + m, :] = W[k * 128:(k + 1) * 128, :]
            gb[:, i * 4 + m, 0] = b

    shared = dict(
        ident=np.eye(128, dtype=np.float32).astype(bf16),
        wxw=np.ascontiguousarray(
            wx_W.reshape(2, 128, H).transpose(1, 0, 2)).astype(bf16),
        wxb=wx_b.reshape(128, 1),
        wr=np.ascontiguousarray(wr.transpose(1, 0, 2)).astype(bf16),
        rinv=np.ascontiguousarray(rinv.transpose(1, 0, 2)).astype(bf16),
        varr=np.ascontiguousarray(varr.transpose(1, 0, 2)).astype(bf16),
        gatb=np.ascontiguousarray(
            gat_b.reshape(DEPTH, 128, 1).transpose(1, 0, 2)),
        gw=gw.astype(bf16),
        gb=gb,
        outw=out_W.astype(bf16),
        outb=np.tile(out_b.reshape(1, OUT_DIM), (128, 1)).astype(np.float32),
    )

    in_maps = []
    for c in range(NCORES):
        cd = cores_data[c]
        perm = cd["perm"]
        xs = np.zeros((NODES_PC, IN_DIM), np.float32)
        valid = perm >= 0
        xs[valid] = x[c * NODES_PC_RAW + perm[valid]]
        m = dict(shared)
        # x layout [128, 2, NODES_PC]: [p, k, n] = x_fm[k*128+p, n]
        xt = np.ascontiguousarray(xs.T).reshape(2, 128, NODES_PC)
        m["x"] = np.ascontiguousarray(xt.transpose(1, 0, 2)).astype(bf16)
        m["idx"] = _wrap_idx(cd["idx"])
        m["oh01"] = cd["oh"]
        m["cnts"] = cd["counts"].reshape(1, -1)
        in_maps.append(m)

    res = run_bass_kernel_spmd(nc, in_maps, core_ids=list(range(NCORES)),
                               **RUN_KWARGS)
    LAST_RESULT = res

    out = np.zeros((N, OUT_DIM), np.float32)
    for c in range(NCORES):
        o = np.asarray(res.results[c]["out"], np.float32)
        perm = cores_data[c]["perm"]
        valid = perm >= 0
        out[c * NODES_PC_RAW + perm[valid]] = o[valid]
    return out


# revision 23
# speedup vs baseline: 1.3224x; 1.2451x over previous
"""GeniePath (GAT breadth + LSTM depth) distributed Trainium2 Bass kernel.

Self-contained: takes FULL unsharded inputs as produced by
reference.setup_inputs(), returns the FULL [N, OUT_DIM] output.

Hardcoded problem shape:
  N=50000 nodes, E=800000 edges, IN_DIM=256, H=128, OUT_DIM=64, DEPTH=3.

Distribution: nodes (and their incoming edges, by dst) are sharded across
8 NeuronCores; weights are replicated. Per GAT layer each core computes the
rotated feature table y = h @ (W R) for its node shard (R is invertible
with first column attn_l, so el = y[:, 0] rides along with gathered rows),
AllGathers the table into every core's HBM, dma_gathers the per-edge source
rows (per (dst-block, src-half) calls whose valid-index counts are loaded
from a per-core table so padding descriptors are skipped), multiplies a
statically preloaded one-hot by the exp-weighted attention factors, and
contracts on the TensorEngine into per-destination aggregates + softmax
denominators. exp(er) factors are partition-replicated via a rank-1 matmul
(keeping the Pool engine free for gathers). The depth LSTM and output
projection are node-parallel. Edge bookkeeping (dst-sorted blocks of 64
nodes in block-major chunk order, low/high split so gather indices fit
int16) is precomputed on the host; all cores share one SPMD graph topology
(per-position chunk counts are maxed across cores).
"""

import numpy as np
import ml_dtypes

N = 50000
E = 800000
IN_DIM = 256
H = 128
OUT_DIM = 64
DEPTH = 3
NEG_SLOPE = 0.2

NCORES = 8
BLK = 64                      # dst nodes per block (one-hot width)
NODES_PC_RAW = N // NCORES    # 6250
NODES_PC = 6272               # = 49*128, padded per-core node count
NTILES = NODES_PC // 128      # 49
NBLK = NODES_PC // BLK        # 98
NTAB = NCORES * NODES_PC      # 50176 rows in the gathered table
LOW_CORES = 5
SPLIT = LOW_CORES * NODES_PC  # 31360 (< 32768 so low indices fit int16)
SEG_MAX_CHUNKS = 32

bf16 = ml_dtypes.bfloat16

_GRAPH = None
_PREP = None
RUN_KWARGS = {}      # test.py may set {"trace": True, "tmpdir": ...}
LAST_RESULT = None


def _preprocess(src, dst):
    """Host-side edge bookkeeping. Returns shared topology + per-core data."""
    src = np.asarray(src, np.int64)
    dst = np.asarray(dst, np.int64)
    core_of = np.minimum(dst // NODES_PC_RAW, NCORES - 1)

    per_core = []
    for c in range(NCORES):
        m = core_of == c
        s_c = src[m]
        d_c = dst[m] - c * NODES_PC_RAW
        lo = s_c // NODES_PC_RAW < LOW_CORES
        blk = d_c // BLK
        nL = np.bincount(blk[lo], minlength=NBLK)
        nH = np.bincount(blk[~lo], minlength=NBLK)
        pL = -(-nL // 128)
        pH = -(-nH // 128)
        per_core.append((s_c, d_c, lo, blk, pL, pH))

    orders = []
    for c in range(NCORES):
        pL, pH = per_core[c][4], per_core[c][5]
        orders.append(np.lexsort((-pL, -(pL + pH))))

    PL = np.zeros(NBLK, np.int64)
    PH = np.zeros(NBLK, np.int64)
    for c in range(NCORES):
        pL, pH = per_core[c][4], per_core[c][5]
        PL = np.maximum(PL, pL[orders[c]])
        PH = np.maximum(PH, pH[orders[c]])
    PL = np.maximum(PL, 1)
    PH = np.maximum(PH, 1)

    # segments: runs of equal (PL, PH), at most SEG_MAX_CHUNKS chunks each
    segments = []
    k = 0
    while k < NBLK:
        pl, ph = int(PL[k]), int(PH[k])
        assert pl + ph <= SEG_MAX_CHUNKS, (pl, ph)
        k2 = k
        while k2 < NBLK and PL[k2] == pl and PH[k2] == ph:
            k2 += 1
        if (pl + ph) * 4 <= SEG_MAX_CHUNKS:
            per_seg = 4
        elif (pl + ph) * 2 <= SEG_MAX_CHUNKS:
            per_seg = 2
        else:
            per_seg = 1
        kk = k
        while kk < k2:
            nb = int(min(per_seg, k2 - kk))
            segments.append((int(kk), nb, pl, ph))
            kk += nb
        k = k2

    # chunk layout: block-major — per block [L chunks | H chunks]
    seg_off = []
    tot = 0
    for (k0, nb, pl, ph) in segments:
        seg_off.append(tot)
        tot += nb * (pl + ph)
    CTOT = tot

    # gather call list: per (segment, block, part) split into <=8-chunk
    # pieces.  Shared across cores and layers.
    calls = []   # (chunk0, nchunks)
    for si, (k0, nb, pl, ph) in enumerate(segments):
        base = seg_off[si]
        for t in range(nb):
            for part, p_ in ((0, pl), (1, ph)):
                coff = base + t * (pl + ph) + (0 if part == 0 else pl)
                done = 0
                while done < p_:
                    n = int(min(8, p_ - done))
                    calls.append((coff + done, n))
                    done += n
    NCALLS = len(calls)

    # node relabeling perms
    perms = []
    for c in range(NCORES):
        perm = np.full(NODES_PC, -1, np.int64)
        order = orders[c]
        for pos in range(NBLK):
            b = order[pos]
            n0 = b * BLK
            n1 = min(n0 + BLK, NODES_PC_RAW)
            cnt = n1 - n0
            perm[pos * BLK: pos * BLK + cnt] = np.arange(n0, n1)
        perms.append(perm)
    inv_all = np.zeros((NCORES, NODES_PC_RAW), np.int64)
    for c in range(NCORES):
        pm = perms[c]
        valid = pm >= 0
        inv_all[c][pm[valid]] = np.nonzero(valid)[0]

    # per-core idx + one-hot + counts arrays in block-major chunk order
    cores_data = []
    for c in range(NCORES):
        s_c, d_c, lo, blk, _, _ = per_core[c]
        order = orders[c]
        idx_all = np.full(CTOT * 128, -1, np.int16)
        oh_all = np.zeros((128, CTOT, BLK), np.float32)
        counts = np.zeros(NCALLS, np.int32)

        sc_core = np.minimum(s_c // NODES_PC_RAW, NCORES - 1)
        s_gid = sc_core * NODES_PC + inv_all[sc_core, s_c - sc_core * NODES_PC_RAW]

        sort_key = np.lexsort((d_c, blk))
        s_gid_s = s_gid[sort_key]
        d_s = d_c[sort_key]
        lo_s = lo[sort_key]
        blk_s = blk[sort_key]
        blk_start = np.searchsorted(blk_s, np.arange(NBLK + 1))

        for si, (k0, nb, pl, ph) in enumerate(segments):
            base = seg_off[si]
            for t in range(nb):
                pos = k0 + t
                b = order[pos]
                sl = slice(blk_start[b], blk_start[b + 1])
                sg = s_gid_s[sl]
                dl = d_s[sl] - b * BLK
                lom = lo_s[sl]
                for part, p_ in ((0, pl), (1, ph)):
                    sel = lom if part == 0 else ~lom
                    sgx = sg[sel]
                    dlx = dl[sel]
                    n = len(sgx)
                    c0 = base + t * (pl + ph) + (0 if part == 0 else pl)
                    cap = 128 * p_
                    assert n <= cap, (c, pos, n, cap)
                    s0 = c0 * 128
                    vals = (sgx - (0 if part == 0 else SPLIT)).astype(np.int16)
                    idx_all[s0: s0 + n] = vals
                    j = np.arange(n)
                    oh_all[j % 128, c0 + j // 128, dlx] = 1.0
                    if n == 0:
                        idx_all[s0] = 0     # keep >=1 valid idx per part
        # per-call valid counts (>=1)
        for i, (c0, nch) in enumerate(calls):
            seg_idx = idx_all[c0 * 128: (c0 + nch) * 128]
            counts[i] = max(1, int((seg_idx >= 0).sum()))
            if (seg_idx >= 0).sum() == 0:
                idx_all[c0 * 128] = 0

        cores_data.append(dict(
            idx=idx_all, oh=oh_all.astype(bf16), counts=counts,
            perm=perms[c],
        ))

    topo = dict(segments=segments, seg_off=seg_off, CTOT=CTOT, calls=calls,
                NCALLS=NCALLS)
    return topo, cores_data


def _wrap_idx(a):
    """dma_gather idx layout: [128, n/16] — 16-wrap, replicated for 8 cores."""
    return np.tile(a.reshape(-1, 16).T.copy(), (8, 1))


def _rotation(a_l):
    """R [H,H] invertible with R[:,0] == a_l; returns (R, Rinv)."""
    a = np.asarray(a_l, np.float64)
    nrm = np.linalg.norm(a)
    v = a / nrm
    s = 1.0 if v[0] >= 0 else -1.0
    w = v.copy()
    w[0] += s
    u = w / np.linalg.norm(w)
    Hh = np.eye(H) - 2.0 * np.outer(u, u)
    R0 = -s * Hh
    Dv = np.ones(H)
    Dv[0] = nrm
    R = R0 * Dv[None, :]
    Rinv = (1.0 / Dv)[:, None] * R0.T
    return R.astype(np.float32), Rinv.astype(np.float32)


def _build_graph(topo, sim_mode=False, no_collective=False):
    import concourse.tile as tile
    from concourse import bacc, mybir

    BF = mybir.dt.bfloat16
    F32 = mybir.dt.float32
    I16 = mybir.dt.int16
    I32 = mybir.dt.int32
    AT = mybir.AluOpType
    AF = mybir.ActivationFunctionType

    segments = topo["segments"]
    seg_off = topo["seg_off"]
    CTOT = topo["CTOT"]
    calls = topo["calls"]
    NCALLS = topo["NCALLS"]

    nc = bacc.Bacc("TRN2", target_bir_lowering=False, debug=False,
                   num_devices=1 if sim_mode else NCORES,
                   num_swdge_queues=4)

    # external tensors (DRAM layout == SBUF layout, partition dim first)
    x_d = nc.dram_tensor("x", [128, 2, NODES_PC], BF, kind="ExternalInput")
    idx_d = nc.dram_tensor("idx", [128, CTOT * 8], I16, kind="ExternalInput")
    oh_d = nc.dram_tensor("oh01", [128, CTOT, BLK], BF, kind="ExternalInput")
    cnt_d = nc.dram_tensor("cnts", [1, NCALLS], I32, kind="ExternalInput")
    ident_d = nc.dram_tensor("ident", [128, 128], BF, kind="ExternalInput")
    wxw_d = nc.dram_tensor("wxw", [128, 2, H], BF, kind="ExternalInput")
    wxb_d = nc.dram_tensor("wxb", [128, 1], F32, kind="ExternalInput")
    wr_d = nc.dram_tensor("wr", [128, DEPTH, H], BF, kind="ExternalInput")
    rinv_d = nc.dram_tensor("rinv", [128, DEPTH, H], BF, kind="ExternalInput")
    varr_d = nc.dram_tensor("varr", [128, DEPTH, 128], BF, kind="ExternalInput")
    gatb_d = nc.dram_tensor("gatb", [128, DEPTH, 1], F32, kind="ExternalInput")
    gw_d = nc.dram_tensor("gw", [128, DEPTH * 8, 128], BF, kind="ExternalInput")
    gb_d = nc.dram_tensor("gb", [128, DEPTH * 4, 1], F32, kind="ExternalInput")
    outw_d = nc.dram_tensor("outw", [128, OUT_DIM], BF, kind="ExternalInput")
    outb_d = nc.dram_tensor("outb", [128, OUT_DIM], F32, kind="ExternalInput")
    out_d = nc.dram_tensor("out", [NODES_PC, OUT_DIM], F32,
                           kind="ExternalOutput")

    NCHUNK = [(i * 512, 512) for i in range(NODES_PC // 512)]
    if NODES_PC % 512:
        NCHUNK.append((NODES_PC - NODES_PC % 512, NODES_PC % 512))

    with tile.TileContext(nc) as tc:
        with (
            tc.tile_pool(name="sb", bufs=1) as sb,
            tc.tile_pool(name="ps", bufs=1, space="PSUM") as psp,
            tc.tile_pool(name="dram", bufs=2, space="DRAM") as dp,
        ):
            def load(dten, shape, dtype):
                t = sb.tile(shape, dtype, name=f"sb_{dten.name}")
                nc.sync.dma_start(t[:], dten.ap())
                return t

            idx_sb = load(idx_d, [128, CTOT * 8], I16)
            cnt_sb = load(cnt_d, [1, NCALLS], I32)
            ident_sb = load(ident_d, [128, 128], BF)
            wxw_sb = load(wxw_d, [128, 2, H], BF)
            wxb_sb = load(wxb_d, [128, 1], F32)
            wr_sb = load(wr_d, [128, DEPTH, H], BF)
            rinv_sb = load(rinv_d, [128, DEPTH, H], BF)
            varr_sb = load(varr_d, [128, DEPTH, 128], BF)
            gatb_sb = load(gatb_d, [128, DEPTH, 1], F32)
            gw_sb = load(gw_d, [128, DEPTH * 8, 128], BF)
            gb_sb = load(gb_d, [128, DEPTH * 4, 1], F32)
            outw_sb = load(outw_d, [128, OUT_DIM], BF)
            outb_sb = load(outb_d, [128, OUT_DIM], F32)
            ones_sb = sb.tile([128, 1], BF, name="ones")
            nc.vector.memset(ones_sb[:], 1.0)

            cst = sb.tile([128, NODES_PC], BF, name="cst")
            mu_bf = sb.tile([128, NODES_PC], BF, name="mu_bf")
            h1 = sb.tile([128, NODES_PC], BF, name="h1")
            h2 = sb.tile([128, NODES_PC], BF, name="h2")
            h0 = sb.tile([128, NODES_PC], BF, tag="h03", bufs=1)
            hcol = [h0, h1, h2, None]  # h3 allocated later from tag h03

            b_rep = sb.tile([128, NODES_PC], BF, name="b_rep")
            b2_rep = sb.tile([128, NODES_PC], BF, name="b2_rep")

            cnt_regs = [nc.gpsimd.alloc_register(f"gcnt{i}") for i in range(8)]

            # pre-zero gather buffers (skipped-pad slots read stale data; it
            # must be finite so 0 * stale == 0 in the aggregation matmul)
            GBUFS = 4
            NSEG = len(segments)
            for _ in range(GBUFS):
                gz = sb.tile([128, SEG_MAX_CHUNKS, 128], BF, tag="gath",
                             bufs=GBUFS)
                nc.vector.memset(gz[:], 0.0)

            def lstm_step(i, h_i, chunks=None):
                for n0, nn in (NCHUNK if chunks is None else chunks):
                    gates = []
                    for m in range(4):
                        ps = psp.tile([128, 512], F32, tag="mmA", bufs=2)
                        for k, rhs in ((0, h_i), (1, mu_bf)):
                            nc.tensor.matmul(ps[:, 0:nn],
                                             gw_sb[:, (i * 2 + k) * 4 + m, :],
                                             rhs[:, n0:n0 + nn],
                                             start=(k == 0), stop=(k == 1))
                        gt = sb.tile([128, 512], F32, tag=f"gate{m}", bufs=1)
                        func = AF.Tanh if m == 3 else AF.Sigmoid
                        nc.scalar.activation(gt[:, 0:nn], ps[:, 0:nn], func,
                                             bias=gb_sb[:, i * 4 + m, :])
                        gates.append(gt)
                    ig, fg, og, ct = gates
                    nc.vector.tensor_mul(ig[:, 0:nn], ig[:, 0:nn], ct[:, 0:nn])
                    nc.vector.tensor_mul(fg[:, 0:nn], fg[:, 0:nn],
                                         cst[:, n0:n0 + nn])
                    nc.vector.tensor_add(cst[:, n0:n0 + nn], ig[:, 0:nn],
                                         fg[:, 0:nn])
                    nc.scalar.activation(ct[:, 0:nn], cst[:, n0:n0 + nn],
                                         AF.Tanh)
                    nc.vector.tensor_mul(mu_bf[:, n0:n0 + nn], og[:, 0:nn],
                                         ct[:, 0:nn])

            def out_proj(chunks):
                # final projection for node columns covered by `chunks`
                for n0, nn in chunks:
                    for t0 in range(n0, n0 + nn, 128):
                        ps = psp.tile([128, 512], F32, tag="mmA", bufs=2)
                        nc.tensor.matmul(ps[:, 0:OUT_DIM],
                                         mu_bf[:, t0:t0 + 128],
                                         outw_sb[:], start=True, stop=True)
                        ob = sb.tile([128, OUT_DIM], F32, tag="otile", bufs=2)
                        nc.vector.tensor_add(ob[:], ps[:, 0:OUT_DIM],
                                             outb_sb[:])
                        nc.vector.tensor_scalar_max(ob[:], ob[:], 0.0)
                        nc.sync.dma_start(out_d.ap()[t0:t0 + 128, :], ob[:])

            # ---- h0 = x @ wx_W + wx_b ----
            for n0, nn in NCHUNK:
                xc = sb.tile([128, 2, 512], BF, tag="xchunk", bufs=2)
                nc.sync.dma_start(xc[:, :, 0:nn], x_d.ap()[:, :, n0:n0 + nn])
                ps = psp.tile([128, 512], F32, tag="mmA", bufs=2)
                for k in range(2):
                    nc.tensor.matmul(ps[:, 0:nn], wxw_sb[:, k, :],
                                     xc[:, k, 0:nn],
                                     start=(k == 0), stop=(k == 1))
                nc.vector.tensor_scalar_add(mu_bf[:, n0:n0 + nn], ps[:, 0:nn],
                                            wxb_sb[:])
                nc.vector.tensor_scalar_add(hcol[0][:, n0:n0 + nn],
                                            ps[:, 0:nn], wxb_sb[:])
            nc.vector.memset(cst[:], 0.0)

            h_cur = hcol[0]

            for layer in range(DEPTH):
                # A: rotated table z = h @ (W R), node-major, to local DRAM
                ztab = dp.tile([NODES_PC, H], BF, name=f"ztab{layer}")
                for t in range(NTILES):
                    ps = psp.tile([128, 512], F32, tag="mmA", bufs=2)
                    nc.tensor.matmul(ps[:, 0:H],
                                     h_cur[:, t * 128:(t + 1) * 128],
                                     wr_sb[:, layer, :], start=True, stop=True)
                    zb = sb.tile([128, H], BF, tag="ztile", bufs=2)
                    nc.scalar.activation(zb[:], ps[:, 0:H], AF.Copy)
                    nc.sync.dma_start(ztab[t * 128:(t + 1) * 128, :], zb[:])

                # C: AllGather the table
                if sim_mode or no_collective:
                    ytab = dp.tile([NTAB, H], BF, name=f"ytab{layer}")
                    for cc in range(NCORES):
                        nc.sync.dma_start(
                            ytab[cc * NODES_PC:(cc + 1) * NODES_PC, :],
                            ztab[:])
                else:
                    ytab = dp.tile([NTAB, H], BF, name=f"ytab{layer}",
                                   addr_space="Shared")
                    nc.gpsimd.collective_compute(
                        "AllGather", AT.bypass,
                        replica_groups=[list(range(NCORES))],
                        ins=[ztab.opt()], outs=[ytab.opt()],
                    )

                # B: b = exp(er), b2 = exp(slope*er), partition-replicated
                # via a rank-1 matmul (varr rows are all equal to attn_r@W);
                # emitted after the collective so it runs underneath it
                for n0, nn in NCHUNK:
                    ps = psp.tile([128, 512], F32, tag="mmA", bufs=2)
                    nc.tensor.matmul(ps[:, 0:nn], varr_sb[:, layer, :],
                                     h_cur[:, n0:n0 + nn],
                                     start=True, stop=True)
                    nc.scalar.activation(b_rep[:, n0:n0 + nn],
                                         ps[:, 0:nn], AF.Exp)
                    nc.scalar.activation(b2_rep[:, n0:n0 + nn],
                                         ps[:, 0:nn], AF.Exp,
                                         scale=NEG_SLOPE)

                # LSTM step for the previous layer overlaps the collective
                if layer >= 1:
                    lstm_step(layer - 1, hcol[layer])

                if layer == DEPTH - 1:
                    h3 = sb.tile([128, NODES_PC], BF, tag="h03", bufs=1,
                                 name="h3")
                    hcol[3] = h3
                h_next = hcol[layer + 1]

                # D: edge phase
                gq = [0]
                call_i = [0]
                lstm_cols = 0     # columns already pushed through final lstm
                pend = []         # deferred normalize/transpose/rinv tails

                def emit_tail(aggd, k, width):
                    deng = sb.tile([128, 1], F32, tag="deng", bufs=2)
                    nc.vector.tensor_scalar_max(deng[0:width],
                                                aggd[0:width, 128:129], 1e-16)
                    rden = sb.tile([128, 1], F32, tag="rden", bufs=2)
                    nc.vector.reciprocal(rden[0:width], deng[0:width])
                    ynorm = sb.tile([128, H], BF, tag="ynorm", bufs=2)
                    nc.vector.tensor_scalar_mul(ynorm[0:width, :],
                                                aggd[0:width, 0:H],
                                                rden[0:width])
                    pt = psp.tile([128, 128], BF, tag="ptr", bufs=1)
                    nc.tensor.transpose(pt[:, 0:width], ynorm[0:width, :],
                                        ident_sb[0:width, 0:width])
                    ptsb = sb.tile([128, 128], BF, tag="ptsb", bufs=2)
                    nc.scalar.activation(ptsb[:, 0:width], pt[:, 0:width],
                                         AF.Copy)
                    pz = psp.tile([128, 128], F32, tag="pz", bufs=1)
                    nc.tensor.matmul(pz[:, 0:width], rinv_sb[:, layer, :],
                                     ptsb[:, 0:width], start=True, stop=True)
                    nc.scalar.activation(
                        h_next[:, k * BLK:k * BLK + width],
                        pz[:, 0:width], AF.Tanh,
                        bias=gatb_sb[:, layer, :])
                    return k * BLK + width

                for si, (k0, nb, pl, ph) in enumerate(segments):
                    c0 = seg_off[si]
                    P = pl + ph
                    nch = nb * P
                    ohs = sb.tile([128, SEG_MAX_CHUNKS, BLK], BF,
                                  tag="oh01", bufs=3)
                    nc.sync.dma_start(ohs[:, 0:nch, :],
                                      oh_d.ap()[:, c0:c0 + nch, :])
                    g = sb.tile([128, SEG_MAX_CHUNKS, 128], BF,
                                tag="gath", bufs=GBUFS)
                    # collect this segment's gather calls, then batch-load
                    # their valid-index counts into registers in one go
                    seg_calls = []
                    for t in range(nb):
                        for part, p_, (tb0, tb1) in (
                            (0, pl, (0, SPLIT)),
                            (1, ph, (SPLIT, NTAB)),
                        ):
                            a0 = t * P + (0 if part == 0 else pl)
                            done = 0
                            while done < p_:
                                n = int(min(8, p_ - done))
                                ci = call_i[0]
                                assert calls[ci] == (c0 + a0 + done, n), (
                                    calls[ci], (c0 + a0 + done, n))
                                call_i[0] += 1
                                seg_calls.append((ci, a0 + done, n, tb0, tb1))
                                done += n
                    ci0 = seg_calls[0][0]
                    ncall = len(seg_calls)
                    assert ncall <= len(cnt_regs), ncall
                    assert seg_calls[-1][0] == ci0 + ncall - 1
                    nc.gpsimd.reg_load(cnt_regs[:ncall],
                                       cnt_sb[0:1, ci0:ci0 + ncall])
                    for (ci, a0, n, tb0, tb1) in seg_calls:
                        nc.gpsimd.dma_gather(
                            out_ap=g[:, a0:a0 + n, :],
                            in_ap=ytab[tb0:tb1, :],
                            idxs_ap=idx_sb[:, (c0 + a0) * 8:
                                           (c0 + a0 + n) * 8],
                            num_idxs=n * 128,
                            num_idxs_reg=cnt_regs[ci - ci0],
                            elem_size=H,
                            queue_num=gq[0],
                        )
                        gq[0] = (gq[0] + 1) % 4

                    a1 = sb.tile([128, SEG_MAX_CHUNKS], BF, tag="a1", bufs=3)
                    nc.scalar.activation(a1[:, 0:nch], g[:, 0:nch, 0], AF.Exp)
                    a2 = sb.tile([128, SEG_MAX_CHUNKS], BF, tag="a2", bufs=3)
                    nc.scalar.activation(a2[:, 0:nch], g[:, 0:nch, 0], AF.Exp,
                                         scale=NEG_SLOPE)

                    m1 = sb.tile([128, SEG_MAX_CHUNKS, BLK], BF,
                                 tag="m1", bufs=3)
                    m2 = sb.tile([128, SEG_MAX_CHUNKS, BLK], BF,
                                 tag="m2", bufs=2)
                    shp = [128, nb, P, BLK]
                    a1v = (a1[:, 0:nch].rearrange("p (nb q) -> p nb q", nb=nb)
                           .unsqueeze(3).broadcast_to(shp))
                    a2v = (a2[:, 0:nch].rearrange("p (nb q) -> p nb q", nb=nb)
                           .unsqueeze(3).broadcast_to(shp))
                    bv = (b_rep[:, k0 * BLK:(k0 + nb) * BLK]
                          .rearrange("p (nb v) -> p nb v", v=BLK)
                          .unsqueeze(2).broadcast_to(shp))
                    b2v = (b2_rep[:, k0 * BLK:(k0 + nb) * BLK]
                           .rearrange("p (nb v) -> p nb v", v=BLK)
                           .unsqueeze(2).broadcast_to(shp))
                    m1_4 = m1[:, 0:nch, :].rearrange(
                        "p (nb q) v -> p nb q v", nb=nb)
                    m2_4 = m2[:, 0:nch, :].rearrange(
                        "p (nb q) v -> p nb q v", nb=nb)
                    nc.vector.tensor_tensor(m1_4, a1v, bv, AT.mult)
                    nc.vector.tensor_tensor(m2_4, a2v, b2v, AT.mult)
                    nc.vector.tensor_tensor(m1[:, 0:nch, :], m1[:, 0:nch, :],
                                            m2[:, 0:nch, :], AT.max)
                    nc.vector.tensor_tensor(m1[:, 0:nch, :], m1[:, 0:nch, :],
                                            ohs[:, 0:nch, :], AT.mult)

                    # aggregate per block; blocks processed in pairs so the
                    # normalize/transpose/rinv tail runs at 128 width; den
                    # accumulates into column 128 of the same PSUM tile and
                    # tails are deferred 2 pairs so in-order engine queues
                    # don't chain segment s's tails into segment s+1's strips
                    t = 0
                    while t < nb:
                        npair = 2 if t + 1 < nb else 1
                        width = 64 * npair
                        aggd = psp.tile([128, 132], F32, tag="agg", bufs=3)
                        for u in range(npair):
                            for q in range(P):
                                ch = (t + u) * P + q
                                st = q == 0
                                sp_ = q == P - 1
                                nc.tensor.matmul(
                                    aggd[u * 64:(u + 1) * 64, 0:H],
                                    m1[:, ch, :], g[:, ch, :],
                                    start=st, stop=sp_)
                                # start=False always: the agg q==0 matmul's
                                # bank-wide has_written clear covers col 128,
                                # so this overwrites on q==0 and accumulates
                                # after — a start here would wipe agg's q==0
                                nc.tensor.matmul(
                                    aggd[u * 64:(u + 1) * 64, 128:129],
                                    m1[:, ch, :], ones_sb[:],
                                    start=False, stop=sp_,
                                    skip_group_check=True)
                        pend.append((aggd, k0 + t, width))
                        if len(pend) > 2:
                            tcols = emit_tail(*pend.pop(0))
                            # interleave final LSTM + output projection with
                            # the last layer's edge phase per 512-col chunk
                            if layer == DEPTH - 1:
                                while lstm_cols + 512 <= tcols:
                                    ck = [(lstm_cols, 512)]
                                    lstm_step(DEPTH - 1, h_next, chunks=ck)
                                    out_proj(ck)
                                    lstm_cols += 512
                        t += npair

                for pe in pend:
                    emit_tail(*pe)
                pend = []
                assert call_i[0] == NCALLS, (call_i[0], NCALLS)
                h_cur = h_next

            while lstm_cols < NODES_PC:
                nn = min(512, NODES_PC - lstm_cols)
                ck = [(lstm_cols, nn)]
                lstm_step(DEPTH - 1, hcol[DEPTH], chunks=ck)
                out_proj(ck)
                lstm_cols += nn

    nc.compile()
    return nc


def kernel(x, src, dst, wx_W, wx_b, gat_W, gat_b, attn_l, attn_r,
           ig_W, ig_b, fg_W, fg_b, og_W, og_b, st_W, st_b, out_W, out_b):
    global _GRAPH, _PREP, LAST_RESULT
    from concourse.bass_utils import run_bass_kernel_spmd

    x = np.asarray(x, np.float32)
    src_i = np.asarray(src, np.int64)
    dst_i = np.asarray(dst, np.int64)

    key = (int(src_i[:100].sum()), int(dst_i[:100].sum()), len(src_i))
    if _PREP is None or _PREP[0] != key:
        topo, cores_data = _preprocess(src_i, dst_i)
        _PREP = (key, topo, cores_data)
    else:
        _, topo, cores_data = _PREP

    if _GRAPH is None:
        _GRAPH = _build_graph(topo)
    nc = _GRAPH

    wx_W = np.asarray(wx_W, np.float32)
    wx_b = np.asarray(wx_b, np.float32)
    gat_W = np.asarray(gat_W, np.float32)
    gat_b = np.asarray(gat_b, np.float32)
    attn_l = np.asarray(attn_l, np.float32)
    attn_r = np.asarray(attn_r, np.float32)
    out_W = np.asarray(out_W, np.float32)
    out_b = np.asarray(out_b, np.float32)

    wr = np.zeros((DEPTH, H, H), np.float32)
    rinv = np.zeros((DEPTH, H, H), np.float32)
    varr = np.zeros((DEPTH, H, 128), np.float32)
    for i in range(DEPTH):
        R, Ri = _rotation(attn_l[i])
        wr[i] = gat_W[i] @ R
        rinv[i] = Ri
        varr[i] = np.repeat((gat_W[i] @ attn_r[i])[:, None], 128, axis=1)

    # gw layout [128, DEPTH*8, 128]: [:, (i*2+k)*4+m, :] = W_m[i][k*128+p, :]
    gw = np.zeros((128, DEPTH * 8, 128), np.float32)
    gb = np.zeros((128, DEPTH * 4, 1), np.float32)
    for i in range(DEPTH):
        for m, (Wm, bm) in enumerate(((ig_W, ig_b), (fg_W, fg_b),
                                      (og_W, og_b), (st_W, st_b))):
            W = np.asarray(Wm, np.float32)[i]
            b = np.asarray(bm, np.float32)[i]
            for k in range(2):
                gw[:, (i * 2 + k) * 4 + m, :] = W[k * 128:(k + 1) * 128, :]
            gb[:, i * 4 + m, 0] = b

    shared = dict(
        ident=np.eye(128, dtype=np.float32).astype(bf16),
        wxw=np.ascontiguousarray(
            wx_W.reshape(2, 128, H).transpose(1, 0, 2)).astype(bf16),
        wxb=wx_b.reshape(128, 1),
        wr=np.ascontiguousarray(wr.transpose(1, 0, 2)).astype(bf16),
        rinv=np.ascontiguousarray(rinv.transpose(1, 0, 2)).astype(bf16),
        varr=np.ascontiguousarray(varr.transpose(1, 0, 2)).astype(bf16),
        gatb=np.ascontiguousarray(
            gat_b.reshape(DEPTH, 128, 1).transpose(1, 0, 2)),
        gw=gw.astype(bf16),
        gb=gb,
        outw=out_W.astype(bf16),
        outb=np.tile(out_b.reshape(1, OUT_DIM), (128, 1)).astype(np.float32),
    )

    in_maps = []
    for c in range(NCORES):
        cd = cores_data[c]
        perm = cd["perm"]
        xs = np.zeros((NODES_PC, IN_DIM), np.float32)
        valid = perm >= 0
        xs[valid] = x[c * NODES_PC_RAW + perm[valid]]
        m = dict(shared)
        # x layout [128, 2, NODES_PC]: [p, k, n] = x_fm[k*128+p, n]
        xt = np.ascontiguousarray(xs.T).reshape(2, 128, NODES_PC)
        m["x"] = np.ascontiguousarray(xt.transpose(1, 0, 2)).astype(bf16)
        m["idx"] = _wrap_idx(cd["idx"])
        m["oh01"] = cd["oh"]
        m["cnts"] = cd["counts"].reshape(1, -1)
        in_maps.append(m)

    res = run_bass_kernel_spmd(nc, in_maps, core_ids=list(range(NCORES)),
                               **RUN_KWARGS)
    LAST_RESULT = res

    out = np.zeros((N, OUT_DIM), np.float32)
    for c in range(NCORES):
        o = np.asarray(res.results[c]["out"], np.float32)
        perm = cores_data[c]["perm"]
        valid = perm >= 0
        out[c * NODES_PC_RAW + perm[valid]] = o[valid]
    return out


# revision 28
# speedup vs baseline: 1.3231x; 1.0005x over previous
"""GeniePath (GAT breadth + LSTM depth) distributed Trainium2 Bass kernel.

Self-contained: takes FULL unsharded inputs as produced by
reference.setup_inputs(), returns the FULL [N, OUT_DIM] output.

Hardcoded problem shape:
  N=50000 nodes, E=800000 edges, IN_DIM=256, H=128, OUT_DIM=64, DEPTH=3.

Distribution: nodes (and their incoming edges, by dst) are sharded across
8 NeuronCores; weights are replicated. Per GAT layer each core computes the
rotated feature table y = h @ (W R) for its node shard (R is invertible
with first column attn_l, so el = y[:, 0] rides along with gathered rows),
AllGathers the table into every core's HBM, dma_gathers the per-edge source
rows (per (dst-block, src-half) calls whose valid-index counts are loaded
from a per-core table so padding descriptors are skipped), multiplies a
statically preloaded one-hot by the exp-weighted attention factors, and
contracts on the TensorEngine into per-destination aggregates + softmax
denominators. exp(er) factors are partition-replicated via a rank-1 matmul
(keeping the Pool engine free for gathers). The depth LSTM and output
projection are node-parallel. Edge bookkeeping (dst-sorted blocks of 64
nodes in block-major chunk order, low/high split so gather indices fit
int16) is precomputed on the host; all cores share one SPMD graph topology
(per-position chunk counts are maxed across cores).
"""

import numpy as np
import ml_dtypes

N = 50000
E = 800000
IN_DIM = 256
H = 128
OUT_DIM = 64
DEPTH = 3
NEG_SLOPE = 0.2

NCORES = 8
BLK = 64                      # dst nodes per block (one-hot width)
NODES_PC_RAW = N // NCORES    # 6250
NODES_PC = 6272               # = 49*128, padded per-core node count
NTILES = NODES_PC // 128      # 49
NBLK = NODES_PC // BLK        # 98
NTAB = NCORES * NODES_PC      # 50176 rows in the gathered table
LOW_CORES = 5
SPLIT = LOW_CORES * NODES_PC  # 31360 (< 32768 so low indices fit int16)
SEG_MAX_CHUNKS = 32

bf16 = ml_dtypes.bfloat16

_GRAPH = None
_PREP = None
RUN_KWARGS = {}      # test.py may set {"trace": True, "tmpdir": ...}
LAST_RESULT = None


def _preprocess(src, dst):
    """Host-side edge bookkeeping. Returns shared topology + per-core data."""
    src = np.asarray(src, np.int64)
    dst = np.asarray(dst, np.int64)
    core_of = np.minimum(dst // NODES_PC_RAW, NCORES - 1)

    per_core = []
    for c in range(NCORES):
        m = core_of == c
        s_c = src[m]
        d_c = dst[m] - c * NODES_PC_RAW
        lo = s_c // NODES_PC_RAW < LOW_CORES
        blk = d_c // BLK
        nL = np.bincount(blk[lo], minlength=NBLK)
        nH = np.bincount(blk[~lo], minlength=NBLK)
        pL = -(-nL // 128)
        pH = -(-nH // 128)
        per_core.append((s_c, d_c, lo, blk, pL, pH))

    orders = []
    for c in range(NCORES):
        pL, pH = per_core[c][4], per_core[c][5]
        orders.append(np.lexsort((-pL, -(pL + pH))))

    PL = np.zeros(NBLK, np.int64)
    PH = np.zeros(NBLK, np.int64)
    for c in range(NCORES):
        pL, pH = per_core[c][4], per_core[c][5]
        PL = np.maximum(PL, pL[orders[c]])
        PH = np.maximum(PH, pH[orders[c]])
    PL = np.maximum(PL, 1)
    PH = np.maximum(PH, 1)

    # segments: runs of equal (PL, PH), at most SEG_MAX_CHUNKS chunks each
    segments = []
    k = 0
    while k < NBLK:
        pl, ph = int(PL[k]), int(PH[k])
        assert pl + ph <= SEG_MAX_CHUNKS, (pl, ph)
        k2 = k
        while k2 < NBLK and PL[k2] == pl and PH[k2] == ph:
            k2 += 1
        if (pl + ph) * 4 <= SEG_MAX_CHUNKS:
            per_seg = 4
        elif (pl + ph) * 2 <= SEG_MAX_CHUNKS:
            per_seg = 2
        else:
            per_seg = 1
        kk = k
        while kk < k2:
            nb = int(min(per_seg, k2 - kk))
            segments.append((int(kk), nb, pl, ph))
            kk += nb
        k = k2

    # chunk layout: block-major — per block [L chunks | H chunks]
    seg_off = []
    tot = 0
    for (k0, nb, pl, ph) in segments:
        seg_off.append(tot)
        tot += nb * (pl + ph)
    CTOT = tot

    # gather call list: per (segment, block, part) split into <=8-chunk
    # pieces.  Shared across cores and layers.
    calls = []   # (chunk0, nchunks)
    for si, (k0, nb, pl, ph) in enumerate(segments):
        base = seg_off[si]
        for t in range(nb):
            for part, p_ in ((0, pl), (1, ph)):
                coff = base + t * (pl + ph) + (0 if part == 0 else pl)
                done = 0
                while done < p_:
                    n = int(min(8, p_ - done))
                    calls.append((coff + done, n))
                    done += n
    NCALLS = len(calls)

    # node relabeling perms
    perms = []
    for c in range(NCORES):
        perm = np.full(NODES_PC, -1, np.int64)
        order = orders[c]
        for pos in range(NBLK):
            b = order[pos]
            n0 = b * BLK
            n1 = min(n0 + BLK, NODES_PC_RAW)
            cnt = n1 - n0
            perm[pos * BLK: pos * BLK + cnt] = np.arange(n0, n1)
        perms.append(perm)
    inv_all = np.zeros((NCORES, NODES_PC_RAW), np.int64)
    for c in range(NCORES):
        pm = perms[c]
        valid = pm >= 0
        inv_all[c][pm[valid]] = np.nonzero(valid)[0]

    # per-core idx + one-hot + counts arrays in block-major chunk order
    cores_data = []
    for c in range(NCORES):
        s_c, d_c, lo, blk, _, _ = per_core[c]
        order = orders[c]
        idx_all = np.full(CTOT * 128, -1, np.int16)
        oh_all = np.zeros((128, CTOT, BLK), np.float32)
        counts = np.zeros(NCALLS, np.int32)

        sc_core = np.minimum(s_c // NODES_PC_RAW, NCORES - 1)
        s_gid = sc_core * NODES_PC + inv_all[sc_core, s_c - sc_core * NODES_PC_RAW]

        sort_key = np.lexsort((d_c, blk))
        s_gid_s = s_gid[sort_key]
        d_s = d_c[sort_key]
        lo_s = lo[sort_key]
        blk_s = blk[sort_key]
        blk_start = np.searchsorted(blk_s, np.arange(NBLK + 1))

        for si, (k0, nb, pl, ph) in enumerate(segments):
            base = seg_off[si]
            for t in range(nb):
                pos = k0 + t
                b = order[pos]
                sl = slice(blk_start[b], blk_start[b + 1])
                sg = s_gid_s[sl]
                dl = d_s[sl] - b * BLK
                lom = lo_s[sl]
                for part, p_ in ((0, pl), (1, ph)):
                    sel = lom if part == 0 else ~lom
                    sgx = sg[sel]
                    dlx = dl[sel]
                    n = len(sgx)
                    c0 = base + t * (pl + ph) + (0 if part == 0 else pl)
                    cap = 128 * p_
                    assert n <= cap, (c, pos, n, cap)
                    s0 = c0 * 128
                    vals = (sgx - (0 if part == 0 else SPLIT)).astype(np.int16)
                    idx_all[s0: s0 + n] = vals
                    j = np.arange(n)
                    oh_all[j % 128, c0 + j // 128, dlx] = 1.0
                    if n == 0:
                        idx_all[s0] = 0     # keep >=1 valid idx per part
        # per-call valid counts (>=1)
        for i, (c0, nch) in enumerate(calls):
            seg_idx = idx_all[c0 * 128: (c0 + nch) * 128]
            counts[i] = max(1, int((seg_idx >= 0).sum()))
            if (seg_idx >= 0).sum() == 0:
                idx_all[c0 * 128] = 0

        cores_data.append(dict(
            idx=idx_all, oh=oh_all.astype(bf16), counts=counts,
            perm=perms[c],
        ))

    topo = dict(segments=segments, seg_off=seg_off, CTOT=CTOT, calls=calls,
                NCALLS=NCALLS)
    return topo, cores_data


def _wrap_idx(a):
    """dma_gather idx layout: [128, n/16] — 16-wrap, replicated for 8 cores."""
    return np.tile(a.reshape(-1, 16).T.copy(), (8, 1))


def _rotation(a_l):
    """R [H,H] invertible with R[:,0] == a_l; returns (R, Rinv)."""
    a = np.asarray(a_l, np.float64)
    nrm = np.linalg.norm(a)
    v = a / nrm
    s = 1.0 if v[0] >= 0 else -1.0
    w = v.copy()
    w[0] += s
    u = w / np.linalg.norm(w)
    Hh = np.eye(H) - 2.0 * np.outer(u, u)
    R0 = -s * Hh
    Dv = np.ones(H)
    Dv[0] = nrm
    R = R0 * Dv[None, :]
    Rinv = (1.0 / Dv)[:, None] * R0.T
    return R.astype(np.float32), Rinv.astype(np.float32)


def _build_graph(topo, sim_mode=False, no_collective=False):
    import concourse.tile as tile
    from concourse import bacc, mybir

    BF = mybir.dt.bfloat16
    F32 = mybir.dt.float32
    I16 = mybir.dt.int16
    I32 = mybir.dt.int32
    AT = mybir.AluOpType
    AF = mybir.ActivationFunctionType

    segments = topo["segments"]
    seg_off = topo["seg_off"]
    CTOT = topo["CTOT"]
    calls = topo["calls"]
    NCALLS = topo["NCALLS"]

    nc = bacc.Bacc("TRN2", target_bir_lowering=False, debug=False,
                   num_devices=1 if sim_mode else NCORES,
                   num_swdge_queues=4)

    # external tensors (DRAM layout == SBUF layout, partition dim first)
    x_d = nc.dram_tensor("x", [128, 2, NODES_PC], BF, kind="ExternalInput")
    idx_d = nc.dram_tensor("idx", [128, CTOT * 8], I16, kind="ExternalInput")
    oh_d = nc.dram_tensor("oh01", [128, CTOT, BLK], BF, kind="ExternalInput")
    cnt_d = nc.dram_tensor("cnts", [1, NCALLS], I32, kind="ExternalInput")
    ident_d = nc.dram_tensor("ident", [128, 128], BF, kind="ExternalInput")
    wxw_d = nc.dram_tensor("wxw", [128, 2, H], BF, kind="ExternalInput")
    wxb_d = nc.dram_tensor("wxb", [128, 1], F32, kind="ExternalInput")
    wr_d = nc.dram_tensor("wr", [128, DEPTH, H], BF, kind="ExternalInput")
    rinv_d = nc.dram_tensor("rinv", [128, DEPTH, H], BF, kind="ExternalInput")
    varr_d = nc.dram_tensor("varr", [128, DEPTH, 128], BF, kind="ExternalInput")
    gatb_d = nc.dram_tensor("gatb", [128, DEPTH, 1], F32, kind="ExternalInput")
    gw_d = nc.dram_tensor("gw", [128, DEPTH * 8, 128], BF, kind="ExternalInput")
    gb_d = nc.dram_tensor("gb", [128, DEPTH * 4, 1], F32, kind="ExternalInput")
    outw_d = nc.dram_tensor("outw", [128, OUT_DIM], BF, kind="ExternalInput")
    outb_d = nc.dram_tensor("outb", [128, OUT_DIM], F32, kind="ExternalInput")
    out_d = nc.dram_tensor("out", [NODES_PC, OUT_DIM], F32,
                           kind="ExternalOutput")

    NCHUNK = [(i * 512, 512) for i in range(NODES_PC // 512)]
    if NODES_PC % 512:
        NCHUNK.append((NODES_PC - NODES_PC % 512, NODES_PC % 512))

    with tile.TileContext(nc) as tc:
        with (
            tc.tile_pool(name="sb", bufs=1) as sb,
            tc.tile_pool(name="ps", bufs=1, space="PSUM") as psp,
            tc.tile_pool(name="dram", bufs=2, space="DRAM") as dp,
        ):
            def load(dten, shape, dtype):
                t = sb.tile(shape, dtype, name=f"sb_{dten.name}")
                nc.sync.dma_start(t[:], dten.ap())
                return t

            idx_sb = load(idx_d, [128, CTOT * 8], I16)
            cnt_sb = load(cnt_d, [1, NCALLS], I32)
            ident_sb = load(ident_d, [128, 128], BF)
            wxw_sb = load(wxw_d, [128, 2, H], BF)
            wxb_sb = load(wxb_d, [128, 1], F32)
            wr_sb = load(wr_d, [128, DEPTH, H], BF)
            rinv_sb = load(rinv_d, [128, DEPTH, H], BF)
            varr_sb = load(varr_d, [128, DEPTH, 128], BF)
            gatb_sb = load(gatb_d, [128, DEPTH, 1], F32)
            gw_sb = load(gw_d, [128, DEPTH * 8, 128], BF)
            gb_sb = load(gb_d, [128, DEPTH * 4, 1], F32)
            outw_sb = load(outw_d, [128, OUT_DIM], BF)
            outb_sb = load(outb_d, [128, OUT_DIM], F32)
            ones_sb = sb.tile([128, 1], BF, name="ones")
            nc.vector.memset(ones_sb[:], 1.0)

            cst = sb.tile([128, NODES_PC], BF, name="cst")
            mu_bf = sb.tile([128, NODES_PC], BF, name="mu_bf")
            h1 = sb.tile([128, NODES_PC], BF, name="h1")
            h2 = sb.tile([128, NODES_PC], BF, name="h2")
            h0 = sb.tile([128, NODES_PC], BF, tag="h03", bufs=1)
            hcol = [h0, h1, h2, None]  # h3 allocated later from tag h03

            b_rep = sb.tile([128, NODES_PC], BF, name="b_rep")
            b2_rep = sb.tile([128, NODES_PC], BF, name="b2_rep")

            cnt_regs = [nc.gpsimd.alloc_register(f"gcnt{i}") for i in range(8)]

            # pre-zero gather buffers (skipped-pad slots read stale data; it
            # must be finite so 0 * stale == 0 in the aggregation matmul)
            GBUFS = 4
            NSEG = len(segments)
            for _ in range(GBUFS):
                gz = sb.tile([128, SEG_MAX_CHUNKS, 128], BF, tag="gath",
                             bufs=GBUFS)
                nc.vector.memset(gz[:], 0.0)

            def lstm_step(i, h_i, chunks=None):
                for n0, nn in (NCHUNK if chunks is None else chunks):
                    gates = []
                    for m in range(4):
                        ps = psp.tile([128, 512], F32, tag="mmA", bufs=2)
                        for k, rhs in ((0, h_i), (1, mu_bf)):
                            nc.tensor.matmul(ps[:, 0:nn],
                                             gw_sb[:, (i * 2 + k) * 4 + m, :],
                                             rhs[:, n0:n0 + nn],
                                             start=(k == 0), stop=(k == 1))
                        gt = sb.tile([128, 512], F32, tag=f"gate{m}", bufs=1)
                        func = AF.Tanh if m == 3 else AF.Sigmoid
                        nc.scalar.activation(gt[:, 0:nn], ps[:, 0:nn], func,
                                             bias=gb_sb[:, i * 4 + m, :])
                        gates.append(gt)
                    ig, fg, og, ct = gates
                    nc.vector.tensor_mul(ig[:, 0:nn], ig[:, 0:nn], ct[:, 0:nn])
                    nc.vector.tensor_mul(fg[:, 0:nn], fg[:, 0:nn],
                                         cst[:, n0:n0 + nn])
                    nc.vector.tensor_add(cst[:, n0:n0 + nn], ig[:, 0:nn],
                                         fg[:, 0:nn])
                    nc.scalar.activation(ct[:, 0:nn], cst[:, n0:n0 + nn],
                                         AF.Tanh)
                    nc.vector.tensor_mul(mu_bf[:, n0:n0 + nn], og[:, 0:nn],
                                         ct[:, 0:nn])

            def out_proj(chunks):
                # final projection for node columns covered by `chunks`
                for n0, nn in chunks:
                    for t0 in range(n0, n0 + nn, 128):
                        ps = psp.tile([128, 512], F32, tag="mmA", bufs=2)
                        nc.tensor.matmul(ps[:, 0:OUT_DIM],
                                         mu_bf[:, t0:t0 + 128],
                                         outw_sb[:], start=True, stop=True)
                        ob = sb.tile([128, OUT_DIM], F32, tag="otile", bufs=2)
                        nc.vector.tensor_add(ob[:], ps[:, 0:OUT_DIM],
                                             outb_sb[:])
                        nc.vector.tensor_scalar_max(ob[:], ob[:], 0.0)
                        nc.sync.dma_start(out_d.ap()[t0:t0 + 128, :], ob[:])

            # ---- h0 = x @ wx_W + wx_b ----
            for n0, nn in NCHUNK:
                xc = sb.tile([128, 2, 512], BF, tag="xchunk", bufs=2)
                nc.sync.dma_start(xc[:, :, 0:nn], x_d.ap()[:, :, n0:n0 + nn])
                ps = psp.tile([128, 512], F32, tag="mmA", bufs=2)
                for k in range(2):
                    nc.tensor.matmul(ps[:, 0:nn], wxw_sb[:, k, :],
                                     xc[:, k, 0:nn],
                                     start=(k == 0), stop=(k == 1))
                nc.vector.tensor_scalar_add(mu_bf[:, n0:n0 + nn], ps[:, 0:nn],
                                            wxb_sb[:])
                nc.vector.tensor_scalar_add(hcol[0][:, n0:n0 + nn],
                                            ps[:, 0:nn], wxb_sb[:])
            nc.vector.memset(cst[:], 0.0)

            h_cur = hcol[0]

            def ztab_tile(zt, lyr, h_src, t):
                # one 128-row tile of the rotated table z = h @ (W R)
                ps = psp.tile([128, 512], F32, tag="mmA", bufs=2)
                nc.tensor.matmul(ps[:, 0:H],
                                 h_src[:, t * 128:(t + 1) * 128],
                                 wr_sb[:, lyr, :], start=True, stop=True)
                zb = sb.tile([128, H], BF, tag="ztile", bufs=2)
                nc.scalar.activation(zb[:], ps[:, 0:H], AF.Copy)
                nc.sync.dma_start(zt[t * 128:(t + 1) * 128, :], zb[:])

            # layer-0 table built up front from h0
            ztab_next = dp.tile([NODES_PC, H], BF, name="ztab0")
            for t in range(NTILES):
                ztab_tile(ztab_next, 0, h_cur, t)

            for layer in range(DEPTH):
                # A: this layer's table was built incrementally by the
                # previous layer's tails (layer 0: right above)
                ztab = ztab_next

                # C: AllGather the table
                if sim_mode or no_collective:
                    ytab = dp.tile([NTAB, H], BF, name=f"ytab{layer}")
                    for cc in range(NCORES):
                        nc.sync.dma_start(
                            ytab[cc * NODES_PC:(cc + 1) * NODES_PC, :],
                            ztab[:])
                else:
                    ytab = dp.tile([NTAB, H], BF, name=f"ytab{layer}",
                                   addr_space="Shared")
                    nc.gpsimd.collective_compute(
                        "AllGather", AT.bypass,
                        replica_groups=[list(range(NCORES))],
                        ins=[ztab.opt()], outs=[ytab.opt()],
                    )

                # B: b = exp(er), b2 = exp(slope*er), partition-replicated
                # via a rank-1 matmul (varr rows are all equal to attn_r@W);
                # emitted after the collective so it runs underneath it
                for n0, nn in NCHUNK:
                    ps = psp.tile([128, 512], F32, tag="mmA", bufs=2)
                    nc.tensor.matmul(ps[:, 0:nn], varr_sb[:, layer, :],
                                     h_cur[:, n0:n0 + nn],
                                     start=True, stop=True)
                    nc.scalar.activation(b_rep[:, n0:n0 + nn],
                                         ps[:, 0:nn], AF.Exp)
                    nc.scalar.activation(b2_rep[:, n0:n0 + nn],
                                         ps[:, 0:nn], AF.Exp,
                                         scale=NEG_SLOPE)

                # LSTM step for the previous layer overlaps the collective
                if layer >= 1:
                    lstm_step(layer - 1, hcol[layer])

                if layer == DEPTH - 1:
                    h3 = sb.tile([128, NODES_PC], BF, tag="h03", bufs=1,
                                 name="h3")
                    hcol[3] = h3
                h_next = hcol[layer + 1]

                # D: edge phase
                gq = [0]
                call_i = [0]
                lstm_cols = 0     # columns already pushed through final lstm
                zt_tiles = [0]    # next-layer ztab tiles emitted so far
                pend = []         # deferred normalize/transpose/rinv tails
                if layer < DEPTH - 1:
                    ztab_next = dp.tile([NODES_PC, H], BF,
                                        name=f"ztab{layer + 1}")

                def emit_tail(aggd, k, width):
                    deng = sb.tile([128, 1], F32, tag="deng", bufs=2)
                    nc.vector.tensor_scalar_max(deng[0:width],
                                                aggd[0:width, 128:129], 1e-16)
                    rden = sb.tile([128, 1], F32, tag="rden", bufs=2)
                    nc.vector.reciprocal(rden[0:width], deng[0:width])
                    ynorm = sb.tile([128, H], BF, tag="ynorm", bufs=2)
                    nc.vector.tensor_scalar_mul(ynorm[0:width, :],
                                                aggd[0:width, 0:H],
                                                rden[0:width])
                    pt = psp.tile([128, 128], BF, tag="ptr", bufs=1)
                    nc.tensor.transpose(pt[:, 0:width], ynorm[0:width, :],
                                        ident_sb[0:width, 0:width])
                    ptsb = sb.tile([128, 128], BF, tag="ptsb", bufs=2)
                    nc.scalar.activation(ptsb[:, 0:width], pt[:, 0:width],
                                         AF.Copy)
                    pz = psp.tile([128, 128], F32, tag="pz", bufs=1)
                    nc.tensor.matmul(pz[:, 0:width], rinv_sb[:, layer, :],
                                     ptsb[:, 0:width], start=True, stop=True)
                    nc.scalar.activation(
                        h_next[:, k * BLK:k * BLK + width],
                        pz[:, 0:width], AF.Tanh,
                        bias=gatb_sb[:, layer, :])
                    # build the next layer's table incrementally as h_next
                    # columns complete; lag one tail (columns < k*BLK) so the
                    # Tensor queue never waits on the tanh just emitted
                    if layer < DEPTH - 1:
                        while (zt_tiles[0] + 1) * 128 <= k * BLK:
                            ztab_tile(ztab_next, layer + 1, h_next,
                                      zt_tiles[0])
                            zt_tiles[0] += 1
                    return k * BLK + width

                for si, (k0, nb, pl, ph) in enumerate(segments):
                    c0 = seg_off[si]
                    P = pl + ph
                    nch = nb * P
                    ohs = sb.tile([128, SEG_MAX_CHUNKS, BLK], BF,
                                  tag="oh01", bufs=3)
                    nc.sync.dma_start(ohs[:, 0:nch, :],
                                      oh_d.ap()[:, c0:c0 + nch, :])
                    g = sb.tile([128, SEG_MAX_CHUNKS, 128], BF,
                                tag="gath", bufs=GBUFS)
                    # collect this segment's gather calls, then batch-load
                    # their valid-index counts into registers in one go
                    seg_calls = []
                    for t in range(nb):
                        for part, p_, (tb0, tb1) in (
                            (0, pl, (0, SPLIT)),
                            (1, ph, (SPLIT, NTAB)),
                        ):
                            a0 = t * P + (0 if part == 0 else pl)
                            done = 0
                            while done < p_:
                                n = int(min(8, p_ - done))
                                ci = call_i[0]
                                assert calls[ci] == (c0 + a0 + done, n), (
                                    calls[ci], (c0 + a0 + done, n))
                                call_i[0] += 1
                                seg_calls.append((ci, a0 + done, n, tb0, tb1))
                                done += n
                    ci0 = seg_calls[0][0]
                    ncall = len(seg_calls)
                    assert ncall <= len(cnt_regs), ncall
                    assert seg_calls[-1][0] == ci0 + ncall - 1
                    nc.gpsimd.reg_load(cnt_regs[:ncall],
                                       cnt_sb[0:1, ci0:ci0 + ncall])
                    for (ci, a0, n, tb0, tb1) in seg_calls:
                        nc.gpsimd.dma_gather(
                            out_ap=g[:, a0:a0 + n, :],
                            in_ap=ytab[tb0:tb1, :],
                            idxs_ap=idx_sb[:, (c0 + a0) * 8:
                                           (c0 + a0 + n) * 8],
                            num_idxs=n * 128,
                            num_idxs_reg=cnt_regs[ci - ci0],
                            elem_size=H,
                            queue_num=gq[0],
                        )
                        gq[0] = (gq[0] + 1) % 4

                    a1 = sb.tile([128, SEG_MAX_CHUNKS], BF, tag="a1", bufs=3)
                    nc.scalar.activation(a1[:, 0:nch], g[:, 0:nch, 0], AF.Exp)
                    a2 = sb.tile([128, SEG_MAX_CHUNKS], BF, tag="a2", bufs=3)
                    nc.scalar.activation(a2[:, 0:nch], g[:, 0:nch, 0], AF.Exp,
                                         scale=NEG_SLOPE)

                    m1 = sb.tile([128, SEG_MAX_CHUNKS, BLK], BF,
                                 tag="m1", bufs=3)
                    m2 = sb.tile([128, SEG_MAX_CHUNKS, BLK], BF,
                                 tag="m2", bufs=2)
                    shp = [128, nb, P, BLK]
                    a1v = (a1[:, 0:nch].rearrange("p (nb q) -> p nb q", nb=nb)
                           .unsqueeze(3).broadcast_to(shp))
                    a2v = (a2[:, 0:nch].rearrange("p (nb q) -> p nb q", nb=nb)
                           .unsqueeze(3).broadcast_to(shp))
                    bv = (b_rep[:, k0 * BLK:(k0 + nb) * BLK]
                          .rearrange("p (nb v) -> p nb v", v=BLK)
                          .unsqueeze(2).broadcast_to(shp))
                    b2v = (b2_rep[:, k0 * BLK:(k0 + nb) * BLK]
                           .rearrange("p (nb v) -> p nb v", v=BLK)
                           .unsqueeze(2).broadcast_to(shp))
                    m1_4 = m1[:, 0:nch, :].rearrange(
                        "p (nb q) v -> p nb q v", nb=nb)
                    m2_4 = m2[:, 0:nch, :].rearrange(
                        "p (nb q) v -> p nb q v", nb=nb)
                    nc.vector.tensor_tensor(m1_4, a1v, bv, AT.mult)
                    nc.vector.tensor_tensor(m2_4, a2v, b2v, AT.mult)
                    nc.vector.tensor_tensor(m1[:, 0:nch, :], m1[:, 0:nch, :],
                                            m2[:, 0:nch, :], AT.max)
                    nc.vector.tensor_tensor(m1[:, 0:nch, :], m1[:, 0:nch, :],
                                            ohs[:, 0:nch, :], AT.mult)

                    # aggregate per block; blocks processed in pairs so the
                    # normalize/transpose/rinv tail runs at 128 width; den
                    # accumulates into column 128 of the same PSUM tile and
                    # tails are deferred 2 pairs so in-order engine queues
                    # don't chain segment s's tails into segment s+1's strips
                    t = 0
                    while t < nb:
                        npair = 2 if t + 1 < nb else 1
                        width = 64 * npair
                        aggd = psp.tile([128, 132], F32, tag="agg", bufs=3)
                        for u in range(npair):
                            for q in range(P):
                                ch = (t + u) * P + q
                                st = q == 0
                                sp_ = q == P - 1
                                nc.tensor.matmul(
                                    aggd[u * 64:(u + 1) * 64, 0:H],
                                    m1[:, ch, :], g[:, ch, :],
                                    start=st, stop=sp_)
                                # start=False always: the agg q==0 matmul's
                                # bank-wide has_written clear covers col 128,
                                # so this overwrites on q==0 and accumulates
                                # after — a start here would wipe agg's q==0
                                nc.tensor.matmul(
                                    aggd[u * 64:(u + 1) * 64, 128:129],
                                    m1[:, ch, :], ones_sb[:],
                                    start=False, stop=sp_,
                                    skip_group_check=True)
                        pend.append((aggd, k0 + t, width))
                        if len(pend) > 2:
                            tcols = emit_tail(*pend.pop(0))
                            # interleave final LSTM + output projection with
                            # the last layer's edge phase per 512-col chunk
                            if layer == DEPTH - 1:
                                while lstm_cols + 512 <= tcols:
                                    ck = [(lstm_cols, 512)]
                                    lstm_step(DEPTH - 1, h_next, chunks=ck)
                                    out_proj(ck)
                                    lstm_cols += 512
                        t += npair

                for pe in pend:
                    emit_tail(*pe)
                pend = []
                if layer < DEPTH - 1:
                    while zt_tiles[0] < NTILES:
                        ztab_tile(ztab_next, layer + 1, h_next, zt_tiles[0])
                        zt_tiles[0] += 1
                assert call_i[0] == NCALLS, (call_i[0], NCALLS)
                h_cur = h_next

            while lstm_cols < NODES_PC:
                nn = min(512, NODES_PC - lstm_cols)
                ck = [(lstm_cols, nn)]
                lstm_step(DEPTH - 1, hcol[DEPTH], chunks=ck)
                out_proj(ck)
                lstm_cols += nn

    nc.compile()
    return nc


def kernel(x, src, dst, wx_W, wx_b, gat_W, gat_b, attn_l, attn_r,
           ig_W, ig_b, fg_W, fg_b, og_W, og_b, st_W, st_b, out_W, out_b):
    global _GRAPH, _PREP, LAST_RESULT
    from concourse.bass_utils import run_bass_kernel_spmd

    x = np.asarray(x, np.float32)
    src_i = np.asarray(src, np.int64)
    dst_i = np.asarray(dst, np.int64)

    key = (int(src_i[:100].sum()), int(dst_i[:100].sum()), len(src_i))
    if _PREP is None or _PREP[0] != key:
        topo, cores_data = _preprocess(src_i, dst_i)
        _PREP = (key, topo, cores_data)
    else:
        _, topo, cores_data = _PREP

    if _GRAPH is None:
        _GRAPH = _build_graph(topo)
    nc = _GRAPH

    wx_W = np.asarray(wx_W, np.float32)
    wx_b = np.asarray(wx_b, np.float32)
    gat_W = np.asarray(gat_W, np.float32)
    gat_b = np.asarray(gat_b, np.float32)
    attn_l = np.asarray(attn_l, np.float32)
    attn_r = np.asarray(attn_r, np.float32)
    out_W = np.asarray(out_W, np.float32)
    out_b = np.asarray(out_b, np.float32)

    wr = np.zeros((DEPTH, H, H), np.float32)
    rinv = np.zeros((DEPTH, H, H), np.float32)
    varr = np.zeros((DEPTH, H, 128), np.float32)
    for i in range(DEPTH):
        R, Ri = _rotation(attn_l[i])
        wr[i] = gat_W[i] @ R
        rinv[i] = Ri
        varr[i] = np.repeat((gat_W[i] @ attn_r[i])[:, None], 128, axis=1)

    # gw layout [128, DEPTH*8, 128]: [:, (i*2+k)*4+m, :] = W_m[i][k*128+p, :]
    gw = np.zeros((128, DEPTH * 8, 128), np.float32)
    gb = np.zeros((128, DEPTH * 4, 1), np.float32)
    for i in range(DEPTH):
        for m, (Wm, bm) in enumerate(((ig_W, ig_b), (fg_W, fg_b),
                                      (og_W, og_b), (st_W, st_b))):
            W = np.asarray(Wm, np.float32)[i]
            b = np.asarray(bm, np.float32)[i]
            for k in range(2):
                gw[:, (i * 2 + k) * 4 + m, :] = W[k * 128:(k + 1) * 128, :]
            gb[:, i * 4 + m, 0] = b

    shared = dict(
        ident=np.eye(128, dtype=np.float32).astype(bf16),
        wxw=np.ascontiguousarray(
            wx_W.reshape(2, 128, H).transpose(1, 0, 2)).astype(bf16),
        wxb=wx_b.reshape(128, 1),
        wr=np.ascontiguousarray(wr.transpose(1, 0, 2)).astype(bf16),
        rinv=np.ascontiguousarray(rinv.transpose(1, 0, 2)).astype(bf16),
        varr=np.ascontiguousarray(varr.transpose(1, 0, 2)).astype(bf16),
        gatb=np.ascontiguousarray(
            gat_b.reshape(DEPTH, 128, 1).transpose(1, 0, 2)),
        gw=gw.astype(bf16),
        gb=gb,
        outw=out_W.astype(bf16),
        outb=np.tile(out_b.reshape(1, OUT_DIM), (128, 1)).astype(np.float32),
    )

    in_maps = []
    for c in range(NCORES):
        cd = cores_data[c]
        perm = cd["perm"]
        xs = np.zeros((NODES_PC, IN_DIM), np.float32)
        valid = perm >= 0
        xs[valid] = x[c * NODES_PC_RAW + perm[valid]]
        m = dict(shared)
        # x layout [128, 2, NODES_PC]: [p, k, n] = x_fm[k*128+p, n]
        xt = np.ascontiguousarray(xs.T).reshape(2, 128, NODES_PC)
        m["x"] = np.ascontiguousarray(xt.transpose(1, 0, 2)).astype(bf16)
        m["idx"] = _wrap_idx(cd["idx"])
        m["oh01"] = cd["oh"]
        m["cnts"] = cd["counts"].reshape(1, -1)
        in_maps.append(m)

    res = run_bass_kernel_spmd(nc, in_maps, core_ids=list(range(NCORES)),
                               **RUN_KWARGS)
    LAST_RESULT = res

    out = np.zeros((N, OUT_DIM), np.float32)
    for c in range(NCORES):
        o = np.asarray(res.results[c]["out"], np.float32)
        perm = cores_data[c]["perm"]
        valid = perm >= 0
        out[c * NODES_PC_RAW + perm[valid]] = o[valid]
    return out


# revision 33
# speedup vs baseline: 1.3709x; 1.0361x over previous
"""GeniePath (GAT breadth + LSTM depth) distributed Trainium2 Bass kernel.

Self-contained: takes FULL unsharded inputs as produced by
reference.setup_inputs(), returns the FULL [N, OUT_DIM] output.

Hardcoded problem shape:
  N=50000 nodes, E=800000 edges, IN_DIM=256, H=128, OUT_DIM=64, DEPTH=3.

Distribution: nodes (and their incoming edges, by dst) are sharded across
8 NeuronCores; weights are replicated. Per GAT layer each core computes the
rotated feature table y = h @ (W R) for its node shard (R is invertible
with first column attn_l, so el = y[:, 0] rides along with gathered rows),
AllGathers the table into every core's HBM, dma_gathers the per-edge source
rows (per (dst-block, src-half) calls whose valid-index counts are loaded
from a per-core table so padding descriptors are skipped), multiplies a
statically preloaded one-hot by the exp-weighted attention factors, and
contracts on the TensorEngine into per-destination aggregates + softmax
denominators. exp(er) factors are partition-replicated via a rank-1 matmul
(keeping the Pool engine free for gathers). The depth LSTM and output
projection are node-parallel. Edge bookkeeping (dst-sorted blocks of 64
nodes in block-major chunk order, low/high split so gather indices fit
int16) is precomputed on the host; all cores share one SPMD graph topology
(per-position chunk counts are maxed across cores).
"""

import numpy as np
import ml_dtypes

N = 50000
E = 800000
IN_DIM = 256
H = 128
OUT_DIM = 64
DEPTH = 3
NEG_SLOPE = 0.2

NCORES = 8
BLK = 64                      # dst nodes per block (one-hot width)
NODES_PC_RAW = N // NCORES    # 6250
NODES_PC = 6272               # = 49*128, padded per-core node count
NTILES = NODES_PC // 128      # 49
NBLK = NODES_PC // BLK        # 98
NTAB = NCORES * NODES_PC      # 50176 rows in the gathered table
LOW_CORES = 5
SPLIT = LOW_CORES * NODES_PC  # 31360 (< 32768 so low indices fit int16)
SEG_MAX_CHUNKS = 32

bf16 = ml_dtypes.bfloat16

_GRAPH = None
_PREP = None
RUN_KWARGS = {}      # test.py may set {"trace": True, "tmpdir": ...}
LAST_RESULT = None


def _preprocess(src, dst):
    """Host-side edge bookkeeping. Returns shared topology + per-core data."""
    src = np.asarray(src, np.int64)
    dst = np.asarray(dst, np.int64)
    core_of = np.minimum(dst // NODES_PC_RAW, NCORES - 1)

    per_core = []
    for c in range(NCORES):
        m = core_of == c
        s_c = src[m]
        d_c = dst[m] - c * NODES_PC_RAW
        lo = s_c // NODES_PC_RAW < LOW_CORES
        blk = d_c // BLK
        nL = np.bincount(blk[lo], minlength=NBLK)
        nH = np.bincount(blk[~lo], minlength=NBLK)
        pL = -(-nL // 128)
        pH = -(-nH // 128)
        per_core.append((s_c, d_c, lo, blk, pL, pH))

    orders = []
    for c in range(NCORES):
        pL, pH = per_core[c][4], per_core[c][5]
        orders.append(np.lexsort((-pL, -(pL + pH))))

    PL = np.zeros(NBLK, np.int64)
    PH = np.zeros(NBLK, np.int64)
    for c in range(NCORES):
        pL, pH = per_core[c][4], per_core[c][5]
        PL = np.maximum(PL, pL[orders[c]])
        PH = np.maximum(PH, pH[orders[c]])
    PL = np.maximum(PL, 1)
    PH = np.maximum(PH, 1)

    # segments: runs of equal (PL, PH), at most SEG_MAX_CHUNKS chunks each
    segments = []
    k = 0
    while k < NBLK:
        pl, ph = int(PL[k]), int(PH[k])
        assert pl + ph <= SEG_MAX_CHUNKS, (pl, ph)
        k2 = k
        while k2 < NBLK and PL[k2] == pl and PH[k2] == ph:
            k2 += 1
        if (pl + ph) * 4 <= SEG_MAX_CHUNKS:
            per_seg = 4
        elif (pl + ph) * 2 <= SEG_MAX_CHUNKS:
            per_seg = 2
        else:
            per_seg = 1
        kk = k
        while kk < k2:
            nb = int(min(per_seg, k2 - kk))
            segments.append((int(kk), nb, pl, ph))
            kk += nb
        k = k2

    # chunk layout: block-major — per block [L chunks | H chunks]
    seg_off = []
    tot = 0
    for (k0, nb, pl, ph) in segments:
        seg_off.append(tot)
        tot += nb * (pl + ph)
    CTOT = tot

    # gather call list: per (segment, block, part) split into <=8-chunk
    # pieces.  Shared across cores and layers.
    calls = []   # (chunk0, nchunks)
    for si, (k0, nb, pl, ph) in enumerate(segments):
        base = seg_off[si]
        for t in range(nb):
            for part, p_ in ((0, pl), (1, ph)):
                coff = base + t * (pl + ph) + (0 if part == 0 else pl)
                done = 0
                while done < p_:
                    n = int(min(8, p_ - done))
                    calls.append((coff + done, n))
                    done += n
    NCALLS = len(calls)

    # node relabeling perms
    perms = []
    for c in range(NCORES):
        perm = np.full(NODES_PC, -1, np.int64)
        order = orders[c]
        for pos in range(NBLK):
            b = order[pos]
            n0 = b * BLK
            n1 = min(n0 + BLK, NODES_PC_RAW)
            cnt = n1 - n0
            perm[pos * BLK: pos * BLK + cnt] = np.arange(n0, n1)
        perms.append(perm)
    inv_all = np.zeros((NCORES, NODES_PC_RAW), np.int64)
    for c in range(NCORES):
        pm = perms[c]
        valid = pm >= 0
        inv_all[c][pm[valid]] = np.nonzero(valid)[0]

    # per-core idx + one-hot + counts arrays in block-major chunk order
    cores_data = []
    for c in range(NCORES):
        s_c, d_c, lo, blk, _, _ = per_core[c]
        order = orders[c]
        idx_all = np.full(CTOT * 128, -1, np.int16)
        oh_all = np.zeros((128, CTOT, BLK), np.float32)
        counts = np.zeros(NCALLS, np.int32)

        sc_core = np.minimum(s_c // NODES_PC_RAW, NCORES - 1)
        s_gid = sc_core * NODES_PC + inv_all[sc_core, s_c - sc_core * NODES_PC_RAW]

        sort_key = np.lexsort((d_c, blk))
        s_gid_s = s_gid[sort_key]
        d_s = d_c[sort_key]
        lo_s = lo[sort_key]
        blk_s = blk[sort_key]
        blk_start = np.searchsorted(blk_s, np.arange(NBLK + 1))

        for si, (k0, nb, pl, ph) in enumerate(segments):
            base = seg_off[si]
            for t in range(nb):
                pos = k0 + t
                b = order[pos]
                sl = slice(blk_start[b], blk_start[b + 1])
                sg = s_gid_s[sl]
                dl = d_s[sl] - b * BLK
                lom = lo_s[sl]
                for part, p_ in ((0, pl), (1, ph)):
                    sel = lom if part == 0 else ~lom
                    sgx = sg[sel]
                    dlx = dl[sel]
                    n = len(sgx)
                    c0 = base + t * (pl + ph) + (0 if part == 0 else pl)
                    cap = 128 * p_
                    assert n <= cap, (c, pos, n, cap)
                    s0 = c0 * 128
                    vals = (sgx - (0 if part == 0 else SPLIT)).astype(np.int16)
                    idx_all[s0: s0 + n] = vals
                    j = np.arange(n)
                    oh_all[j % 128, c0 + j // 128, dlx] = 1.0
                    if n == 0:
                        idx_all[s0] = 0     # keep >=1 valid idx per part
        # per-call valid counts (>=1)
        for i, (c0, nch) in enumerate(calls):
            seg_idx = idx_all[c0 * 128: (c0 + nch) * 128]
            counts[i] = max(1, int((seg_idx >= 0).sum()))
            if (seg_idx >= 0).sum() == 0:
                idx_all[c0 * 128] = 0

        cores_data.append(dict(
            idx=idx_all, oh=oh_all.astype(bf16), counts=counts,
            perm=perms[c],
        ))

    topo = dict(segments=segments, seg_off=seg_off, CTOT=CTOT, calls=calls,
                NCALLS=NCALLS)
    return topo, cores_data


def _wrap_idx(a):
    """dma_gather idx layout: [128, n/16] — 16-wrap, replicated for 8 cores."""
    return np.tile(a.reshape(-1, 16).T.copy(), (8, 1))


def _rotation(a_l):
    """R [H,H] invertible with R[:,0] == a_l; returns (R, Rinv)."""
    a = np.asarray(a_l, np.float64)
    nrm = np.linalg.norm(a)
    v = a / nrm
    s = 1.0 if v[0] >= 0 else -1.0
    w = v.copy()
    w[0] += s
    u = w / np.linalg.norm(w)
    Hh = np.eye(H) - 2.0 * np.outer(u, u)
    R0 = -s * Hh
    Dv = np.ones(H)
    Dv[0] = nrm
    R = R0 * Dv[None, :]
    Rinv = (1.0 / Dv)[:, None] * R0.T
    return R.astype(np.float32), Rinv.astype(np.float32)


def _build_graph(topo, sim_mode=False, no_collective=False):
    import concourse.tile as tile
    from concourse import bacc, mybir

    BF = mybir.dt.bfloat16
    F32 = mybir.dt.float32
    I16 = mybir.dt.int16
    I32 = mybir.dt.int32
    AT = mybir.AluOpType
    AF = mybir.ActivationFunctionType

    segments = topo["segments"]
    seg_off = topo["seg_off"]
    CTOT = topo["CTOT"]
    calls = topo["calls"]
    NCALLS = topo["NCALLS"]

    nc = bacc.Bacc("TRN2", target_bir_lowering=False, debug=False,
                   num_devices=1 if sim_mode else NCORES,
                   num_swdge_queues=4)

    # external tensors (DRAM layout == SBUF layout, partition dim first)
    x_d = nc.dram_tensor("x", [128, 2, NODES_PC], BF, kind="ExternalInput")
    idx_d = nc.dram_tensor("idx", [128, CTOT * 8], I16, kind="ExternalInput")
    oh_d = nc.dram_tensor("oh01", [128, CTOT, BLK], BF, kind="ExternalInput")
    cnt_d = nc.dram_tensor("cnts", [1, NCALLS], I32, kind="ExternalInput")
    ident_d = nc.dram_tensor("ident", [128, 128], BF, kind="ExternalInput")
    wxw_d = nc.dram_tensor("wxw", [128, 2, H], BF, kind="ExternalInput")
    wxb_d = nc.dram_tensor("wxb", [128, 1], F32, kind="ExternalInput")
    wr_d = nc.dram_tensor("wr", [128, DEPTH, H], BF, kind="ExternalInput")
    rinv_d = nc.dram_tensor("rinv", [128, DEPTH, H], BF, kind="ExternalInput")
    varr_d = nc.dram_tensor("varr", [128, DEPTH, 128], BF, kind="ExternalInput")
    gatb_d = nc.dram_tensor("gatb", [128, DEPTH, 1], F32, kind="ExternalInput")
    gw_d = nc.dram_tensor("gw", [128, DEPTH * 8, 128], BF, kind="ExternalInput")
    gb_d = nc.dram_tensor("gb", [128, DEPTH * 4, 1], F32, kind="ExternalInput")
    outw_d = nc.dram_tensor("outw", [128, OUT_DIM], BF, kind="ExternalInput")
    outb_d = nc.dram_tensor("outb", [128, OUT_DIM], F32, kind="ExternalInput")
    out_d = nc.dram_tensor("out", [NODES_PC, OUT_DIM], F32,
                           kind="ExternalOutput")

    NCHUNK = [(i * 512, 512) for i in range(NODES_PC // 512)]
    if NODES_PC % 512:
        NCHUNK.append((NODES_PC - NODES_PC % 512, NODES_PC % 512))

    with tile.TileContext(nc) as tc:
        with (
            tc.tile_pool(name="sb", bufs=1) as sb,
            tc.tile_pool(name="ps", bufs=1, space="PSUM") as psp,
            tc.tile_pool(name="dram", bufs=2, space="DRAM") as dp,
        ):
            def load(dten, shape, dtype):
                t = sb.tile(shape, dtype, name=f"sb_{dten.name}")
                nc.sync.dma_start(t[:], dten.ap())
                return t

            idx_sb = load(idx_d, [128, CTOT * 8], I16)
            cnt_sb = load(cnt_d, [1, NCALLS], I32)
            ident_sb = load(ident_d, [128, 128], BF)
            wxw_sb = load(wxw_d, [128, 2, H], BF)
            wxb_sb = load(wxb_d, [128, 1], F32)
            wr_sb = load(wr_d, [128, DEPTH, H], BF)
            rinv_sb = load(rinv_d, [128, DEPTH, H], BF)
            varr_sb = load(varr_d, [128, DEPTH, 128], BF)
            gatb_sb = load(gatb_d, [128, DEPTH, 1], F32)
            gw_sb = load(gw_d, [128, DEPTH * 8, 128], BF)
            gb_sb = load(gb_d, [128, DEPTH * 4, 1], F32)
            outw_sb = load(outw_d, [128, OUT_DIM], BF)
            outb_sb = load(outb_d, [128, OUT_DIM], F32)
            ones_sb = sb.tile([128, 1], BF, name="ones")
            nc.vector.memset(ones_sb[:], 1.0)

            cst = sb.tile([128, NODES_PC], BF, name="cst")
            mu_bf = sb.tile([128, NODES_PC], BF, name="mu_bf")
            h1 = sb.tile([128, NODES_PC], BF, name="h1")
            h2 = sb.tile([128, NODES_PC], BF, name="h2")
            h0 = sb.tile([128, NODES_PC], BF, tag="h03", bufs=1)
            hcol = [h0, h1, h2, None]  # h3 allocated later from tag h03

            b_rep = sb.tile([128, NODES_PC], BF, name="b_rep")
            b2_rep = sb.tile([128, NODES_PC], BF, name="b2_rep")

            cnt_regs = [nc.gpsimd.alloc_register(f"gcnt{i}") for i in range(8)]

            # pre-zero gather buffers (skipped-pad slots read stale data; it
            # must be finite so 0 * stale == 0 in the aggregation matmul)
            GBUFS = 5
            NSEG = len(segments)
            for _ in range(GBUFS):
                gz = sb.tile([128, SEG_MAX_CHUNKS, 128], BF, tag="gath",
                             bufs=GBUFS)
                nc.vector.memset(gz[:], 0.0)

            def lstm_step(i, h_i, chunks=None):
                for n0, nn in (NCHUNK if chunks is None else chunks):
                    gates = []
                    for m in range(4):
                        ps = psp.tile([128, 512], F32, tag="mmA", bufs=2)
                        for k, rhs in ((0, h_i), (1, mu_bf)):
                            nc.tensor.matmul(ps[:, 0:nn],
                                             gw_sb[:, (i * 2 + k) * 4 + m, :],
                                             rhs[:, n0:n0 + nn],
                                             start=(k == 0), stop=(k == 1))
                        gt = sb.tile([128, 512], F32, tag=f"gate{m}", bufs=1)
                        func = AF.Tanh if m == 3 else AF.Sigmoid
                        nc.scalar.activation(gt[:, 0:nn], ps[:, 0:nn], func,
                                             bias=gb_sb[:, i * 4 + m, :])
                        gates.append(gt)
                    ig, fg, og, ct = gates
                    nc.vector.tensor_mul(ig[:, 0:nn], ig[:, 0:nn], ct[:, 0:nn])
                    nc.vector.tensor_mul(fg[:, 0:nn], fg[:, 0:nn],
                                         cst[:, n0:n0 + nn])
                    nc.vector.tensor_add(cst[:, n0:n0 + nn], ig[:, 0:nn],
                                         fg[:, 0:nn])
                    nc.scalar.activation(ct[:, 0:nn], cst[:, n0:n0 + nn],
                                         AF.Tanh)
                    nc.vector.tensor_mul(mu_bf[:, n0:n0 + nn], og[:, 0:nn],
                                         ct[:, 0:nn])

            def out_proj(chunks):
                # final projection for node columns covered by `chunks`
                for n0, nn in chunks:
                    for t0 in range(n0, n0 + nn, 128):
                        ps = psp.tile([128, 512], F32, tag="mmA", bufs=2)
                        nc.tensor.matmul(ps[:, 0:OUT_DIM],
                                         mu_bf[:, t0:t0 + 128],
                                         outw_sb[:], start=True, stop=True)
                        ob = sb.tile([128, OUT_DIM], F32, tag="otile", bufs=2)
                        nc.vector.tensor_add(ob[:], ps[:, 0:OUT_DIM],
                                             outb_sb[:])
                        nc.vector.tensor_scalar_max(ob[:], ob[:], 0.0)
                        nc.sync.dma_start(out_d.ap()[t0:t0 + 128, :], ob[:])

            # ---- h0 = x @ wx_W + wx_b ----
            for n0, nn in NCHUNK:
                xc = sb.tile([128, 2, 512], BF, tag="xchunk", bufs=2)
                nc.sync.dma_start(xc[:, :, 0:nn], x_d.ap()[:, :, n0:n0 + nn])
                ps = psp.tile([128, 512], F32, tag="mmA", bufs=2)
                for k in range(2):
                    nc.tensor.matmul(ps[:, 0:nn], wxw_sb[:, k, :],
                                     xc[:, k, 0:nn],
                                     start=(k == 0), stop=(k == 1))
                nc.vector.tensor_scalar_add(mu_bf[:, n0:n0 + nn], ps[:, 0:nn],
                                            wxb_sb[:])
                nc.vector.tensor_scalar_add(hcol[0][:, n0:n0 + nn],
                                            ps[:, 0:nn], wxb_sb[:])
            nc.vector.memset(cst[:], 0.0)

            h_cur = hcol[0]

            def ztab_tiles(zt, lyr, h_src, t0, nt):
                # up to 4 128-row tiles of the rotated table z = h @ (W R),
                # batched into one PSUM bank / activation / DMA
                ps = psp.tile([128, 512], F32, tag="mmA", bufs=2)
                for g in range(nt):
                    t = t0 + g
                    nc.tensor.matmul(ps[:, g * H:(g + 1) * H],
                                     h_src[:, t * 128:(t + 1) * 128],
                                     wr_sb[:, lyr, :],
                                     start=(g == 0), stop=(g == nt - 1),
                                     skip_group_check=g > 0)
                zb = sb.tile([128, 4, H], BF, tag="ztile", bufs=2)
                nc.scalar.activation(zb[:, 0:nt, :], ps[:, 0:nt * H], AF.Copy)
                out_ap = (zt[t0 * 128:(t0 + nt) * 128, :]
                          .rearrange("(g p) h -> p g h", g=nt))
                nc.sync.dma_start(out_ap, zb[:, 0:nt, :])

            # layer-0 table built up front from h0
            ztab_next = dp.tile([NODES_PC, H], BF, name="ztab0")
            for t in range(0, NTILES, 4):
                ztab_tiles(ztab_next, 0, h_cur, t, min(4, NTILES - t))

            for layer in range(DEPTH):
                # A: this layer's table was built incrementally by the
                # previous layer's tails (layer 0: right above)
                ztab = ztab_next

                # C: AllGather the table
                if sim_mode or no_collective:
                    ytab = dp.tile([NTAB, H], BF, name=f"ytab{layer}")
                    for cc in range(NCORES):
                        nc.sync.dma_start(
                            ytab[cc * NODES_PC:(cc + 1) * NODES_PC, :],
                            ztab[:])
                else:
                    ytab = dp.tile([NTAB, H], BF, name=f"ytab{layer}",
                                   addr_space="Shared")
                    nc.gpsimd.collective_compute(
                        "AllGather", AT.bypass,
                        replica_groups=[list(range(NCORES))],
                        ins=[ztab.opt()], outs=[ytab.opt()],
                    )

                # B: b = exp(er), b2 = exp(slope*er), partition-replicated
                # via a rank-1 matmul (varr rows are all equal to attn_r@W);
                # emitted after the collective so it runs underneath it
                for n0, nn in NCHUNK:
                    ps = psp.tile([128, 512], F32, tag="mmA", bufs=2)
                    nc.tensor.matmul(ps[:, 0:nn], varr_sb[:, layer, :],
                                     h_cur[:, n0:n0 + nn],
                                     start=True, stop=True)
                    nc.scalar.activation(b_rep[:, n0:n0 + nn],
                                         ps[:, 0:nn], AF.Exp)
                    nc.scalar.activation(b2_rep[:, n0:n0 + nn],
                                         ps[:, 0:nn], AF.Exp,
                                         scale=NEG_SLOPE)

                # LSTM step for the previous layer overlaps the collective
                if layer >= 1:
                    lstm_step(layer - 1, hcol[layer])

                if layer == DEPTH - 1:
                    h3 = sb.tile([128, NODES_PC], BF, tag="h03", bufs=1,
                                 name="h3")
                    hcol[3] = h3
                h_next = hcol[layer + 1]

                # D: edge phase
                gq = [0]
                call_i = [0]
                lstm_cols = 0     # columns already pushed through final lstm
                zt_tiles = [0]    # next-layer ztab tiles emitted so far
                pend = []         # deferred normalize/transpose/rinv tails
                if layer < DEPTH - 1:
                    ztab_next = dp.tile([NODES_PC, H], BF,
                                        name=f"ztab{layer + 1}")

                def emit_tail(aggd, k, width):
                    deng = sb.tile([128, 1], F32, tag="deng", bufs=2)
                    nc.vector.tensor_scalar_max(deng[0:width],
                                                aggd[0:width, 128:129], 1e-16)
                    rden = sb.tile([128, 1], F32, tag="rden", bufs=2)
                    nc.vector.reciprocal(rden[0:width], deng[0:width])
                    ynorm = sb.tile([128, H], BF, tag="ynorm", bufs=2)
                    nc.vector.tensor_scalar_mul(ynorm[0:width, :],
                                                aggd[0:width, 0:H],
                                                rden[0:width])
                    pt = psp.tile([128, 128], BF, tag="ptr", bufs=1)
                    nc.tensor.transpose(pt[:, 0:width], ynorm[0:width, :],
                                        ident_sb[0:width, 0:width])
                    ptsb = sb.tile([128, 128], BF, tag="ptsb", bufs=2)
                    nc.scalar.activation(ptsb[:, 0:width], pt[:, 0:width],
                                         AF.Copy)
                    pz = psp.tile([128, 128], F32, tag="pz", bufs=1)
                    nc.tensor.matmul(pz[:, 0:width], rinv_sb[:, layer, :],
                                     ptsb[:, 0:width], start=True, stop=True)
                    nc.scalar.activation(
                        h_next[:, k * BLK:k * BLK + width],
                        pz[:, 0:width], AF.Tanh,
                        bias=gatb_sb[:, layer, :])
                    # build the next layer's table incrementally as h_next
                    # columns complete; lag one tail (columns < k*BLK) so the
                    # Tensor queue never waits on the tanh just emitted
                    if layer < DEPTH - 1:
                        while (zt_tiles[0] + 4) * 128 <= k * BLK:
                            ztab_tiles(ztab_next, layer + 1, h_next,
                                       zt_tiles[0], 4)
                            zt_tiles[0] += 4
                    return k * BLK + width

                for si, (k0, nb, pl, ph) in enumerate(segments):
                    c0 = seg_off[si]
                    P = pl + ph
                    nch = nb * P
                    ohs = sb.tile([128, SEG_MAX_CHUNKS, BLK], BF,
                                  tag="oh01", bufs=3)
                    nc.sync.dma_start(ohs[:, 0:nch, :],
                                      oh_d.ap()[:, c0:c0 + nch, :])
                    g = sb.tile([128, SEG_MAX_CHUNKS, 128], BF,
                                tag="gath", bufs=GBUFS)
                    # collect this segment's gather calls, then batch-load
                    # their valid-index counts into registers in one go
                    seg_calls = []
                    for t in range(nb):
                        for part, p_, (tb0, tb1) in (
                            (0, pl, (0, SPLIT)),
                            (1, ph, (SPLIT, NTAB)),
                        ):
                            a0 = t * P + (0 if part == 0 else pl)
                            done = 0
                            while done < p_:
                                n = int(min(8, p_ - done))
                                ci = call_i[0]
                                assert calls[ci] == (c0 + a0 + done, n), (
                                    calls[ci], (c0 + a0 + done, n))
                                call_i[0] += 1
                                seg_calls.append((ci, a0 + done, n, tb0, tb1))
                                done += n
                    ci0 = seg_calls[0][0]
                    ncall = len(seg_calls)
                    assert ncall <= len(cnt_regs), ncall
                    assert seg_calls[-1][0] == ci0 + ncall - 1
                    nc.gpsimd.reg_load(cnt_regs[:ncall],
                                       cnt_sb[0:1, ci0:ci0 + ncall])
                    for (ci, a0, n, tb0, tb1) in seg_calls:
                        nc.gpsimd.dma_gather(
                            out_ap=g[:, a0:a0 + n, :],
                            in_ap=ytab[tb0:tb1, :],
                            idxs_ap=idx_sb[:, (c0 + a0) * 8:
                                           (c0 + a0 + n) * 8],
                            num_idxs=n * 128,
                            num_idxs_reg=cnt_regs[ci - ci0],
                            elem_size=H,
                            queue_num=gq[0],
                        )
                        gq[0] = (gq[0] + 1) % 4

                    a1 = sb.tile([128, SEG_MAX_CHUNKS], BF, tag="a1", bufs=3)
                    nc.scalar.activation(a1[:, 0:nch], g[:, 0:nch, 0], AF.Exp)
                    a2 = sb.tile([128, SEG_MAX_CHUNKS], BF, tag="a2", bufs=3)
                    nc.scalar.activation(a2[:, 0:nch], g[:, 0:nch, 0], AF.Exp,
                                         scale=NEG_SLOPE)

                    m1 = sb.tile([128, SEG_MAX_CHUNKS, BLK], BF,
                                 tag="m1", bufs=3)
                    m2 = sb.tile([128, SEG_MAX_CHUNKS, BLK], BF,
                                 tag="m2", bufs=2)
                    shp = [128, nb, P, BLK]
                    a1v = (a1[:, 0:nch].rearrange("p (nb q) -> p nb q", nb=nb)
                           .unsqueeze(3).broadcast_to(shp))
                    a2v = (a2[:, 0:nch].rearrange("p (nb q) -> p nb q", nb=nb)
                           .unsqueeze(3).broadcast_to(shp))
                    bv = (b_rep[:, k0 * BLK:(k0 + nb) * BLK]
                          .rearrange("p (nb v) -> p nb v", v=BLK)
                          .unsqueeze(2).broadcast_to(shp))
                    b2v = (b2_rep[:, k0 * BLK:(k0 + nb) * BLK]
                           .rearrange("p (nb v) -> p nb v", v=BLK)
                           .unsqueeze(2).broadcast_to(shp))
                    m1_4 = m1[:, 0:nch, :].rearrange(
                        "p (nb q) v -> p nb q v", nb=nb)
                    m2_4 = m2[:, 0:nch, :].rearrange(
                        "p (nb q) v -> p nb q v", nb=nb)
                    nc.vector.tensor_tensor(m1_4, a1v, bv, AT.mult)
                    nc.vector.tensor_tensor(m2_4, a2v, b2v, AT.mult)
                    nc.vector.tensor_tensor(m1[:, 0:nch, :], m1[:, 0:nch, :],
                                            m2[:, 0:nch, :], AT.max)
                    nc.vector.tensor_tensor(m1[:, 0:nch, :], m1[:, 0:nch, :],
                                            ohs[:, 0:nch, :], AT.mult)

                    # aggregate per block; blocks processed in pairs so the
                    # normalize/transpose/rinv tail runs at 128 width; den
                    # accumulates into column 128 of the same PSUM tile and
                    # tails are deferred 2 pairs so in-order engine queues
                    # don't chain segment s's tails into segment s+1's strips
                    t = 0
                    while t < nb:
                        npair = 2 if t + 1 < nb else 1
                        width = 64 * npair
                        aggd = psp.tile([128, 132], F32, tag="agg", bufs=3)
                        for u in range(npair):
                            for q in range(P):
                                ch = (t + u) * P + q
                                st = q == 0
                                sp_ = q == P - 1
                                nc.tensor.matmul(
                                    aggd[u * 64:(u + 1) * 64, 0:H],
                                    m1[:, ch, :], g[:, ch, :],
                                    start=st, stop=sp_)
                                # start=False always: the agg q==0 matmul's
                                # bank-wide has_written clear covers col 128,
                                # so this overwrites on q==0 and accumulates
                                # after — a start here would wipe agg's q==0
                                nc.tensor.matmul(
                                    aggd[u * 64:(u + 1) * 64, 128:129],
                                    m1[:, ch, :], ones_sb[:],
                                    start=False, stop=sp_,
                                    skip_group_check=True)
                        pend.append((aggd, k0 + t, width))
                        if len(pend) > 2:
                            tcols = emit_tail(*pend.pop(0))
                            # interleave final LSTM + output projection with
                            # the last layer's edge phase per 512-col chunk
                            if layer == DEPTH - 1:
                                while lstm_cols + 512 <= tcols:
                                    ck = [(lstm_cols, 512)]
                                    lstm_step(DEPTH - 1, h_next, chunks=ck)
                                    out_proj(ck)
                                    lstm_cols += 512
                        t += npair

                for pe in pend:
                    emit_tail(*pe)
                pend = []
                if layer < DEPTH - 1:
                    while zt_tiles[0] < NTILES:
                        nt = min(4, NTILES - zt_tiles[0])
                        ztab_tiles(ztab_next, layer + 1, h_next,
                                   zt_tiles[0], nt)
                        zt_tiles[0] += nt
                assert call_i[0] == NCALLS, (call_i[0], NCALLS)
                h_cur = h_next

            while lstm_cols < NODES_PC:
                nn = min(512, NODES_PC - lstm_cols)
                ck = [(lstm_cols, nn)]
                lstm_step(DEPTH - 1, hcol[DEPTH], chunks=ck)
                out_proj(ck)
                lstm_cols += nn

    nc.compile()
    return nc


def kernel(x, src, dst, wx_W, wx_b, gat_W, gat_b, attn_l, attn_r,
           ig_W, ig_b, fg_W, fg_b, og_W, og_b, st_W, st_b, out_W, out_b):
    global _GRAPH, _PREP, LAST_RESULT
    from concourse.bass_utils import run_bass_kernel_spmd

    x = np.asarray(x, np.float32)
    src_i = np.asarray(src, np.int64)
    dst_i = np.asarray(dst, np.int64)

    key = (int(src_i[:100].sum()), int(dst_i[:100].sum()), len(src_i))
    if _PREP is None or _PREP[0] != key:
        topo, cores_data = _preprocess(src_i, dst_i)
        _PREP = (key, topo, cores_data)
    else:
        _, topo, cores_data = _PREP

    if _GRAPH is None:
        _GRAPH = _build_graph(topo)
    nc = _GRAPH

    wx_W = np.asarray(wx_W, np.float32)
    wx_b = np.asarray(wx_b, np.float32)
    gat_W = np.asarray(gat_W, np.float32)
    gat_b = np.asarray(gat_b, np.float32)
    attn_l = np.asarray(attn_l, np.float32)
    attn_r = np.asarray(attn_r, np.float32)
    out_W = np.asarray(out_W, np.float32)
    out_b = np.asarray(out_b, np.float32)

    wr = np.zeros((DEPTH, H, H), np.float32)
    rinv = np.zeros((DEPTH, H, H), np.float32)
    varr = np.zeros((DEPTH, H, 128), np.float32)
    for i in range(DEPTH):
        R, Ri = _rotation(attn_l[i])
        wr[i] = gat_W[i] @ R
        rinv[i] = Ri
        varr[i] = np.repeat((gat_W[i] @ attn_r[i])[:, None], 128, axis=1)

    # gw layout [128, DEPTH*8, 128]: [:, (i*2+k)*4+m, :] = W_m[i][k*128+p, :]
    gw = np.zeros((128, DEPTH * 8, 128), np.float32)
    gb = np.zeros((128, DEPTH * 4, 1), np.float32)
    for i in range(DEPTH):
        for m, (Wm, bm) in enumerate(((ig_W, ig_b), (fg_W, fg_b),
                                      (og_W, og_b), (st_W, st_b))):
            W = np.asarray(Wm, np.float32)[i]
            b = np.asarray(bm, np.float32)[i]
            for k in range(2):
                gw[:, (i * 2 + k) * 4 + m, :] = W[k * 128:(k + 1) * 128, :]
            gb[:, i * 4 + m, 0] = b

    shared = dict(
        ident=np.eye(128, dtype=np.float32).astype(bf16),
        wxw=np.ascontiguousarray(
            wx_W.reshape(2, 128, H).transpose(1, 0, 2)).astype(bf16),
        wxb=wx_b.reshape(128, 1),
        wr=np.ascontiguousarray(wr.transpose(1, 0, 2)).astype(bf16),
        rinv=np.ascontiguousarray(rinv.transpose(1, 0, 2)).astype(bf16),
        varr=np.ascontiguousarray(varr.transpose(1, 0, 2)).astype(bf16),
        gatb=np.ascontiguousarray(
            gat_b.reshape(DEPTH, 128, 1).transpose(1, 0, 2)),
        gw=gw.astype(bf16),
        gb=gb,
        outw=out_W.astype(bf16),
        outb=np.tile(out_b.reshape(1, OUT_DIM), (128, 1)).astype(np.float32),
    )

    in_maps = []
    for c in range(NCORES):
        cd = cores_data[c]
        perm = cd["perm"]
        xs = np.zeros((NODES_PC, IN_DIM), np.float32)
        valid = perm >= 0
        xs[valid] = x[c * NODES_PC_RAW + perm[valid]]
        m = dict(shared)
        # x layout [128, 2, NODES_PC]: [p, k, n] = x_fm[k*128+p, n]
        xt = np.ascontiguousarray(xs.T).reshape(2, 128, NODES_PC)
        m["x"] = np.ascontiguousarray(xt.transpose(1, 0, 2)).astype(bf16)
        m["idx"] = _wrap_idx(cd["idx"])
        m["oh01"] = cd["oh"]
        m["cnts"] = cd["counts"].reshape(1, -1)
        in_maps.append(m)

    res = run_bass_kernel_spmd(nc, in_maps, core_ids=list(range(NCORES)),
                               **RUN_KWARGS)
    LAST_RESULT = res

    out = np.zeros((N, OUT_DIM), np.float32)
    for c in range(NCORES):
        o = np.asarray(res.results[c]["out"], np.float32)
        perm = cores_data[c]["perm"]
        valid = perm >= 0
        out[c * NODES_PC_RAW + perm[valid]] = o[valid]
    return out
